# revision 13
# baseline (speedup 1.0000x reference)
"""Multi-head attention (B=16, N=577, C=768, H=12) on 8 TRN2 NeuronCores.

Strategy: pure data parallelism over batch (2 images per core, no
collectives). Per core, everything is computed "channels-on-partitions"
(transposed) so that no on-device transposes are ever needed:

  qkT[outc, tok]  = qkv_wT-tiles.T @ xT          (q scaled 1/8 + bias on evict)
  V[tok, outc]    = xT-tiles.T @ qkv_wT          (natural layout; col 64 of
                                                  each 65-block = 1; v_bias is
                                                  folded into proj bias on host
                                                  since sum_k attn[t,k] = 1)
  S^T[nk, nq]     = K^T-tiles.T @ Q^T            (K=64 contraction)
  E^T             = exp(S^T) * exp(relbT)        (host precomputes exp of the
                                                  transposed rel-pos bias; no
                                                  max subtraction -- logits are
                                                  bounded ~|7| for this problem)
  O'^T[65, nq]    = [V_h | 1]-tiles.T @ E^T      (row 64 = softmax denominator)
  O^T             = O'^T[0:64] * bcast(1/O'^T[64])   (recip + o-mult read PSUM
                                                  directly; no evict copies)
  out^T[co, tok]  = projT-tiles.T @ O^T + proj_b (+ proj_w @ v_bias)

Round-2 performance notes (see git-less history in the repo docstrings):
  - S matmuls emitted 512p0, 512p1, rump0, rump1 so the two heads' row-group
    tiles (0,0)/(64,0) run concurrently in the PE array (MMs are strict FIFO
    in issue order; pairing the same-F ones makes both overlap)
  - the normalize chain reads PSUM directly: reciprocal_approx_fast takes the
    denominator row straight out of the AV psum, the o-mult tensor_tensor
    takes O' straight out of psum (one psum operand + one sbuf operand is
    legal), so the old dr/ost copies (~28us of DVE) are gone
  - weights/x live in per-DMA-region tiles (wq0/wk0/wqr/wkr/wv, xa/xb) so
    preamble DMAs have no same-tile semaphore coupling, and the preamble is
    issued on BOTH hwdge queues (sync + scalar) -- x on sync, weights on
    scalar -- so the first matmul starts ~10us instead of ~15.4us
  - v_bias folded into proj bias on host (attn rows sum to 1), so the V
    eviction is a plain copy and vb/vbr tiles + broadcast disappear
  - fill rebalanced: qk chunks in iters 0-3 (evicts split ACT/DVE), V b1 +
    projT dma in 4-5, two b0-proj chunks in each of 6-11 (DVE evicts except
    the last iteration) so the PE never idles long enough to trip the HAM
    MID window (re-throttle to K=4/8)
  - pair-0's S steps interleave with the b0 V groups in the lead-in
  - output is written bf16 (host casts back to f32): halves the output DMA
    bytes; adds ~0.1% relative error against a 2e-2 gate
  - fp8 was evaluated and rejected: any fp8 quantization in the attention
    path contributes its full ~2-3% relative error to the output, busting
    the 2e-2 gate

Host side pre-transposes all inputs (and converts to bf16) and transposes
the output back. PSUM accumulation is f32 throughout.
"""
import numpy as np
import ml_dtypes

B, N, C, H, HD = 16, 577, 768, 12, 64
NCORES = 8
BPC = B // NCORES          # batches per core: 2
NT = BPC * N               # tokens per core: 1154
P = 128

# token-free-dim chunks over NT (matmul free dim <= 512 for f32 psum)
TFREE = [(0, 512), (512, 512), (1024, 130)]
# nk (key token) tiles over N
NKT = [(0, 128), (128, 128), (256, 128), (384, 128), (512, 65)]

_CACHE = {}

# Debug toggles for the fused normalize chain
RECIP_FROM_PSUM = False
OMULT_FROM_PSUM = True
# Emit sim-only memsets of never-read psum regions so CoreSim's
# uninitialized-read checker passes (simtest.py sets this True).
SIM_MEMSET = False


def _build():
    import concourse.tile as tile
    from concourse import bacc, mybir

    bf16 = mybir.dt.bfloat16
    f32 = mybir.dt.float32
    Alu = mybir.AluOpType
    Act = mybir.ActivationFunctionType

    nc = bacc.Bacc(
        "TRN2",
        target_bir_lowering=False,
        debug=False,
        enable_asserts=False,
        num_devices=NCORES,
    )
    xT = nc.dram_tensor("xT", [C, NT], bf16, kind="ExternalInput").ap()
    wqkvT = nc.dram_tensor("wqkvT", [C, 3 * C], bf16, kind="ExternalInput").ap()
    qbias = nc.dram_tensor("qbias", [P, 6], f32, kind="ExternalInput").ap()
    relbT = nc.dram_tensor("relbT", [H, 640, N], bf16, kind="ExternalInput").ap()
    projT = nc.dram_tensor("projT", [C, C], bf16, kind="ExternalInput").ap()
    pbias = nc.dram_tensor("pbias", [P, 6], f32, kind="ExternalInput").ap()
    out = nc.dram_tensor("out", [C, NT], bf16, kind="ExternalOutput").ap()

    with tile.TileContext(nc) as tc:
        with (
            tc.tile_pool(name="persist", bufs=1) as pp,
            tc.tile_pool(name="relb", bufs=2) as relp,
            tc.tile_pool(name="st", bufs=2) as stp,
            tc.tile_pool(name="dn", bufs=4) as dnp,
            tc.tile_pool(name="oev", bufs=3) as oevp,
            tc.tile_pool(name="pss", bufs=2, space="PSUM") as ps_s,
            tc.tile_pool(name="psrump", bufs=1, space="PSUM") as ps_r,
            tc.tile_pool(name="pso", bufs=2, space="PSUM") as ps_o,
        ):
            # ---------------- Phase A: load weights / constants ----------
            # One tile per DMA region so there is no same-tile write
            # coupling; x chunks go on the sync hwdge queue, weights +
            # biases on the scalar hwdge queue (parallel transfer rings).
            qb = pp.tile([P, 6], f32, tag="qb", name="qb")
            pb = pp.tile([P, 6], f32, tag="pb", name="pb")
            xa = pp.tile([P, 6, 512], bf16, tag="xa", name="xa")        # tok 0:512
            xb = pp.tile([P, 6, NT - 512], bf16, tag="xb", name="xb")   # tok 512:NT
            wq0 = pp.tile([P, 6, P], bf16, tag="wq0", name="wq0")       # qkv col 0:128
            wk0 = pp.tile([P, 6, P], bf16, tag="wk0", name="wk0")       # col 768:896
            wqr = pp.tile([P, 6, 5 * P], bf16, tag="wqr", name="wqr")   # col 128:768
            wkr = pp.tile([P, 6, 5 * P], bf16, tag="wkr", name="wkr")   # col 896:1536
            wv = pp.tile([P, 6, C], bf16, tag="wv", name="wv")          # col 1536:2304
            ptall = pp.tile([P, 6, C], bf16, tag="ptall", name="ptall")
            pt = [ptall[:, i, :] for i in range(6)]

            def xsl(ki, f0, fsz):
                # x slice for contraction row-block ki, tokens [f0, f0+fsz)
                if f0 + fsz <= 512:
                    return xa[:, ki, f0 : f0 + fsz]
                assert f0 >= 512
                return xb[:, ki, f0 - 512 : f0 - 512 + fsz]

            def wsl(t, ki):
                # qkv weight columns [128*t, 128*(t+1)) for row-block ki
                if t == 0:
                    return wq0[:, ki, :]
                if t == 6:
                    return wk0[:, ki, :]
                if t < 6:
                    return wqr[:, ki, P * (t - 1) : P * t]
                return wkr[:, ki, P * (t - 7) : P * (t - 6)]

            # sync queue: x (the long pole for the first matmuls)
            nc.sync.dma_start(
                xa[:, :, :], xT[:, 0:512].rearrange("(i p) n -> p i n", p=P)
            )
            nc.sync.dma_start(
                xb[:, :, :], xT[:, 512:NT].rearrange("(i p) n -> p i n", p=P)
            )
            # scalar queue: weights + biases, ordered by first use
            nc.scalar.dma_start(
                wq0[:, :, :], wqkvT[:, 0:P].rearrange("(i p) n -> p i n", p=P)
            )
            nc.scalar.dma_start(
                wk0[:, :, :], wqkvT[:, 6 * P : 7 * P].rearrange("(i p) n -> p i n", p=P)
            )
            nc.scalar.dma_start(qb[:], qbias[:])
            nc.scalar.dma_start(pb[:], pbias[:])
            nc.scalar.dma_start(
                wv[:, :, :], wqkvT[:, 2 * C : 3 * C].rearrange("(i p) n -> p i n", p=P)
            )
            nc.scalar.dma_start(
                wqr[:, :, :], wqkvT[:, P : 6 * P].rearrange("(i p) n -> p i n", p=P)
            )
            nc.scalar.dma_start(
                wkr[:, :, :],
                wqkvT[:, 7 * P : 2 * C].rearrange("(i p) n -> p i n", p=P),
            )

            # ---------------- persistent result tiles ----------------------
            # qk[t] for t in 0..11: [128, NT] bf16, outc block t (q: 0-5, k: 6-11)
            qk = []
            for t in range(12):
                qk.append(pp.tile([P, NT], bf16, tag=f"qk{t}", name=f"qk{t}"))
            # o[t]: [128, NT] bf16 -- O^T assembled for the projection
            o = []
            for t in range(6):
                o.append(pp.tile([P, NT], bf16, tag=f"o{t}", name=f"o{t}"))
            v = [[None] * 5 for _ in range(BPC)]

            def qk_chunk_emit(t, f0, fsz, eng):
                ps = ps_s.tile([P, 1024], f32, tag="ps_s", name="psmm")
                for ki in range(6):
                    nc.tensor.matmul(
                        ps[:, 0:fsz],
                        wsl(t, ki),
                        xsl(ki, f0, fsz),
                        start=(ki == 0),
                        stop=(ki == 5),
                    )
                if t < 6:  # q: scale 1/8 + bias (pre-scaled on host)
                    if eng == "act":
                        nc.scalar.activation(
                            qk[t][:, f0 : f0 + fsz], ps[:, 0:fsz], Act.Identity,
                            bias=qb[:, t : t + 1], scale=0.125,
                        )
                    else:
                        nc.vector.tensor_scalar(
                            qk[t][:, f0 : f0 + fsz], ps[:, 0:fsz], 0.125,
                            qb[:, t : t + 1], op0=Alu.mult, op1=Alu.add,
                        )
                else:  # k: plain copy (k bias is zero)
                    if eng == "act":
                        nc.scalar.copy(qk[t][:, f0 : f0 + fsz], ps[:, 0:fsz])
                    else:
                        nc.vector.tensor_copy(qk[t][:, f0 : f0 + fsz], ps[:, 0:fsz])

            def qk_group(t):
                for (f0, fsz) in TFREE:
                    qk_chunk_emit(t, f0, fsz, "act")

            def v_half_emit(bb, j, half):
                # V projection (natural layout) for batch bb, token tile j.
                # v[bb][j]: [nksz, 780] bf16, 12 head-blocks of [V_h(64) | 1]
                nk0, nksz = NKT[j]
                if half == 0:
                    v[bb][j] = pp.tile(
                        [P, 12 * 65], bf16, tag=f"v{bb}_{j}", name=f"v{bb}_{j}"
                    )
                vt = v[bb][j]
                v3 = vt[:, :].rearrange("p (h w) -> p h w", w=65)
                if half == 0:
                    nc.gpsimd.memset(v3[:, :, 64:65], 1.0)
                tok0 = bb * N + nk0
                f0 = 384 * half
                ps = ps_s.tile([P, 1024], f32, tag="ps_s", name="psmm")
                for ki in range(6):
                    nc.tensor.matmul(
                        ps[0:nksz, 0:384],
                        xsl(ki, tok0, nksz),
                        wv[:, ki, f0 : f0 + 384],
                        start=(ki == 0),
                        stop=(ki == 5),
                    )
                ps3 = ps[0:nksz, 0:384].rearrange("p (h w) -> p h w", w=64)
                nc.vector.tensor_copy(
                    v3[0:nksz, 6 * half : 6 * half + 6, 0:64], ps3[:, :, :]
                )

            def proj_group(t, f0, fsz, eng):
                ps = ps_s.tile([P, 1024], f32, tag="ps_s", name="psmm")
                for ki in range(6):
                    nc.tensor.matmul(
                        ps[:, 0:fsz],
                        pt[ki][:, P * t : P * (t + 1)],
                        o[ki][:, f0 : f0 + fsz],
                        start=(ki == 0),
                        stop=(ki == 5),
                    )
                ot = oevp.tile([P, 512], bf16, tag="oev", name="oev")
                if eng == "act":
                    nc.scalar.activation(
                        ot[:, 0:fsz], ps[:, 0:fsz], Act.Identity,
                        bias=pb[:, t : t + 1],
                    )
                else:
                    nc.vector.tensor_scalar(
                        ot[:, 0:fsz], ps[:, 0:fsz], pb[:, t : t + 1], None,
                        op0=Alu.add,
                    )
                nc.sync.dma_start(out[P * t : P * (t + 1), f0 : f0 + fsz], ot[:, 0:fsz])

            # -------------- pipelined attention phases ---------------------
            # staP/rbaP: [128, 2*5N] bf16, head parity pr at cols [pr*5N, (pr+1)*5N)
            def s_prologue(b, h0):
                # rel-bias DMAs + tile allocs for the pair (h0, h0+1)
                rbaP = relp.tile([P, 10 * N], bf16, tag="rba", name="rba")
                staP = stp.tile([P, 10 * N], bf16, tag="sta", name="sta")
                r4 = rbaP[:, :].rearrange("p (h j q) -> p h j q", h=2, q=N)
                nc.sync.dma_start(
                    r4[:, :, :, :],
                    relbT[h0 : h0 + 2, 0:640, :].rearrange(
                        "h (j p) q -> p h j q", p=P
                    ),
                )
                rump = ps_r.tile([P, 1024], f32, tag="rump", name="rump")
                if SIM_MEMSET:
                    # rows 65:128 of the j=4 rump chunks are never written by
                    # the S matmuls (nksz=65) but the strided epilogue exp
                    # reads them (the exp'd garbage is itself never read);
                    # zero them so CoreSim's uninitialized-read check passes
                    rz = rump[65:128, :].rearrange("p (h q) -> p h q", h=2)
                    nc.gpsimd.memset(rz[:, :, 260:325], 0.0)
                return staP, rbaP, rump

            def s_step(b, h0, st, j):
                # S matmuls + one pair-merged exp evict for nk tile j.
                # Emission order pairs the two 512-wide MMs (row groups
                # (0,0) / (64,0) -> concurrent), then the two rump MMs.
                staP, rbaP, rump = st
                qt = h0 // 2
                nk0, nksz = NKT[j]
                ps = ps_s.tile([P, 1024], f32, tag="ps_s", name="pss")
                lk = {}
                for hh in (h0, h0 + 1):
                    pr = hh % 2
                    qoff = pr * 64
                    lk[hh] = qk[6 + qt][
                        qoff : qoff + 64, b * N + nk0 : b * N + nk0 + nksz
                    ]
                    nc.tensor.matmul(
                        ps[0:nksz, 512 * pr : 512 * pr + 512],
                        lk[hh],
                        qk[qt][qoff : qoff + 64, b * N : b * N + 512],
                        start=True,
                        stop=True,
                    )
                for hh in (h0, h0 + 1):
                    pr = hh % 2
                    qoff = pr * 64
                    nc.tensor.matmul(
                        rump[0:nksz, 512 * pr + 65 * j : 512 * pr + 65 * j + 65],
                        lk[hh],
                        qk[qt][qoff : qoff + 64, b * N + 512 : b * N + N],
                        start=True,
                        stop=True,
                    )
                # one exp for both heads' 512-chunks (adjacent psum banks)
                s2 = staP[:, :].rearrange("p (h q) -> p h q", h=2)
                p2 = ps[:, :].rearrange("p (h q) -> p h q", h=2)
                nc.scalar.activation(
                    s2[0:nksz, :, N * j : N * j + 512],
                    p2[0:nksz, :, :],
                    Act.Exp,
                )

            def s_epilogue(st):
                # one strided exp for all ten 65-wide rumps of the pair
                # (rows 65:128 of the j=4 chunks hold garbage -- never read)
                staP, rbaP, rump = st
                s3 = staP[:, :].rearrange("p (h j q) -> p h j q", h=2, q=N)
                r3 = rump[:, :].rearrange("p (h q) -> p h q", h=2)[
                    :, :, 0:325
                ].rearrange("p h (j q) -> p h j q", q=65)
                nc.scalar.activation(
                    s3[:, :, 0:5, 512:577], r3[:, :, :, :], Act.Exp
                )

            def mult_phase(st):
                # multiplicative rel-bias, both heads at once, on DVE.
                # The j=4 block (cols 4N:5N) only has 65 valid nk rows for
                # queries 0:512 (the exp never writes rows 65:128 there), so
                # it gets its own row-restricted op.
                staP, rbaP, rump = st
                s2 = staP[:, :].rearrange("p (h q) -> p h q", h=2)
                r2 = rbaP[:, :].rearrange("p (h q) -> p h q", h=2)
                nc.vector.tensor_tensor(
                    s2[:, :, 0 : 3 * N], s2[:, :, 0 : 3 * N], r2[:, :, 0 : 3 * N],
                    op=Alu.mult,
                )
                nc.vector.tensor_tensor(
                    s2[:, :, 3 * N : 4 * N], s2[:, :, 3 * N : 4 * N],
                    r2[:, :, 3 * N : 4 * N],
                    op=Alu.mult,
                )
                nc.vector.tensor_tensor(
                    s2[0:65, :, 4 * N : 5 * N], s2[0:65, :, 4 * N : 5 * N],
                    r2[0:65, :, 4 * N : 5 * N],
                    op=Alu.mult,
                )

            def av_alloc(hh):
                # pass-1 psum: [65, 512] (one bank per head, both heads live
                # until the o-mult reads them straight out of PSUM)
                ps1 = ps_o.tile([65, 512], f32, tag="o", name="pso1")
                return ps1

            def av_step(b, hh, ps1, staP, j):
                pr = hh % 2
                nk0, nksz = NKT[j]
                lv = v[b][j][0:nksz, 65 * hh : 65 * hh + 65]
                nc.tensor.matmul(
                    ps1[0:65, 0:512],
                    lv,
                    staP[0:nksz, 5 * N * pr + N * j : 5 * N * pr + N * j + 512],
                    start=(j == 0),
                    stop=(j == 4),
                )

            def av_evict1(ps1):
                # pass-1 psum covers ALL nk for queries 0:512, so the A-half
                # denominator is final here: reciprocal of the denominator row
                # + broadcast for queries 0:512 run while pass 2 computes
                rr = dnp.tile([1, N], f32, tag="rr", name="rr")
                if RECIP_FROM_PSUM:
                    nc.vector.reciprocal_approx_fast(
                        rr[0:1, 0:512], ps1[64:65, 0:512]
                    )
                else:
                    dr = dnp.tile([1, N], f32, tag="dr", name="dr")
                    nc.vector.tensor_copy(dr[0:1, 0:512], ps1[64:65, 0:512])
                    nc.vector.reciprocal_approx_fast(rr[0:1, 0:512], dr[0:1, 0:512])
                rb = dnp.tile([64, N], f32, tag="rbb", name="rbb")
                nc.gpsimd.partition_broadcast(rb[0:64, 0:512], rr[0:1, 0:512])
                return rr, rb

            def av_pass2(b, h0, avs, drs, staP):
                # nq rump (cols 512:577) for both heads, one ps_r-pool tile
                # (bank-split h0/h1); then finish: recip from psum + broadcast
                ps2 = ps_r.tile([P, 1024], f32, tag="rump", name="pso2")
                fins = {}
                for hh in (h0, h0 + 1):
                    pr = hh % 2
                    for j, (nk0, nksz) in enumerate(NKT):
                        lv = v[b][j][0:nksz, 65 * hh : 65 * hh + 65]
                        nc.tensor.matmul(
                            ps2[0:65, 512 * pr : 512 * pr + 65],
                            lv,
                            staP[0:nksz, 5 * N * pr + N * j + 512 : 5 * N * pr + N * j + N],
                            start=(j == 0),
                            stop=(j == 4),
                        )
                for hh in (h0, h0 + 1):
                    pr = hh % 2
                    rr, rb = drs[hh]
                    if RECIP_FROM_PSUM:
                        nc.vector.reciprocal_approx_fast(
                            rr[0:1, 512:577], ps2[64:65, 512 * pr : 512 * pr + 65]
                        )
                    else:
                        dr2 = dnp.tile([1, N], f32, tag="dr2", name="dr2")
                        nc.vector.tensor_copy(
                            dr2[0:1, 512:577], ps2[64:65, 512 * pr : 512 * pr + 65]
                        )
                        nc.vector.reciprocal_approx_fast(
                            rr[0:1, 512:577], dr2[0:1, 512:577]
                        )
                    nc.gpsimd.partition_broadcast(rb[0:64, 512:577], rr[0:1, 512:577])
                    fins[hh] = (avs[hh], ps2, rb)
                return fins

            def omult(b, hh, fin, half):
                # O = O'/denominator, reading O' straight out of PSUM
                ps1, ps2, rb = fin
                pr = hh % 2
                qt = hh // 2
                qoff = (hh % 2) * 64
                if half == 0:
                    if not OMULT_FROM_PSUM:
                        ost = dnp.tile([64, 512], f32, tag="ost", name="ost")
                        nc.vector.tensor_copy(ost[0:64, 0:512], ps1[0:64, 0:512])
                        src = ost[0:64, 0:512]
                    else:
                        src = ps1[0:64, 0:512]
                    nc.vector.tensor_tensor(
                        o[qt][qoff : qoff + 64, b * N : b * N + 512],
                        src,
                        rb[0:64, 0:512],
                        op=Alu.mult,
                    )
                else:
                    if not OMULT_FROM_PSUM:
                        ost = dnp.tile([64, 512], f32, tag="ost2", name="ost2")
                        nc.vector.tensor_copy(
                            ost[0:64, 0:65], ps2[0:64, 512 * pr : 512 * pr + 65]
                        )
                        src = ost[0:64, 0:65]
                    else:
                        src = ps2[0:64, 512 * pr : 512 * pr + 65]
                    nc.vector.tensor_tensor(
                        o[qt][qoff : qoff + 64, b * N + 512 : b * N + N],
                        src,
                        rb[0:64, 512:577],
                        op=Alu.mult,
                    )

            # proj token chunks, batch-aligned
            PFREE0 = [(0, 512), (512, 65)]           # batch 0 tokens
            PFREE1 = [(577, 512), (1089, 65)]        # batch 1 tokens

            # ------------- pipelined emission schedule ---------------------
            def qk_chunk(t, ci, eng):
                def f():
                    f0, fsz = TFREE[ci]
                    qk_chunk_emit(t, f0, fsz, eng)
                return f

            def v_half(bb, j, half):
                def f():
                    v_half_emit(bb, j, half)
                return f

            def pj(t, ci, eng="dve"):
                def f():
                    f0, fsz = PFREE0[ci]
                    proj_group(t, f0, fsz, eng)
                return f

            def pt_dma():
                def f():
                    nc.sync.dma_start(
                        ptall[:, :, :], projT[:, :].rearrange("(i p) n -> p i n", p=P)
                    )
                return f

            fill = {
                0: [qk_chunk(2, 0, "act"), qk_chunk(8, 0, "dve"),
                    qk_chunk(2, 1, "act"), qk_chunk(8, 1, "dve"),
                    qk_chunk(2, 2, "act"), qk_chunk(8, 2, "dve")],
                1: [qk_chunk(3, 0, "act"), qk_chunk(9, 0, "dve"),
                    qk_chunk(3, 1, "act"), qk_chunk(9, 1, "dve"),
                    qk_chunk(3, 2, "act"), qk_chunk(9, 2, "dve")],
                2: [qk_chunk(4, 0, "act"), qk_chunk(10, 0, "dve"),
                    qk_chunk(4, 1, "act"), qk_chunk(10, 1, "dve"),
                    qk_chunk(4, 2, "act"), qk_chunk(10, 2, "dve")],
                3: [qk_chunk(5, 0, "act"), qk_chunk(11, 0, "dve"),
                    qk_chunk(5, 1, "act"), qk_chunk(11, 1, "dve"),
                    qk_chunk(5, 2, "act"), qk_chunk(11, 2, "dve")],
                4: [pt_dma(), v_half(1, 0, 0), v_half(1, 0, 1),
                    v_half(1, 1, 0), v_half(1, 1, 1), v_half(1, 2, 0)],
                5: [v_half(1, 2, 1), v_half(1, 3, 0), v_half(1, 3, 1),
                    v_half(1, 4, 0), v_half(1, 4, 1)],
                6: [pj(0, 0), pj(0, 1)],
                7: [pj(1, 0), pj(1, 1)],
                8: [pj(2, 0), pj(2, 1)],
                9: [pj(3, 0), pj(3, 1)],
                10: [pj(4, 0), pj(4, 1)],
                11: [pj(5, 0, "act"), pj(5, 1, "act")],
            }

            # ---- lead-in: qk 0/6, then pair-0 S interleaved with b0 V ----
            qk_group(0)
            qk_group(6)
            pairs = [(b, h0) for b in range(BPC) for h0 in range(0, 12, 2)]
            st = s_prologue(*pairs[0])
            for j in range(5):
                s_step(*pairs[0], st, j)
                v_half_emit(0, j, 0)
                v_half_emit(0, j, 1)
            s_epilogue(st)
            qk_group(1)
            qk_group(7)
            mult_phase(st)
            cur = {0: st}
            for k in range(12):
                b, h0 = pairs[k]
                staP = cur[k][0]
                fq = list(fill[k])
                avs = {hh: av_alloc(hh) for hh in (h0, h0 + 1)}
                if k + 1 < 12:
                    nb, nh0 = pairs[k + 1]
                    cur[k + 1] = s_prologue(nb, nh0)
                    # interleave: S of pair k+1 with AV of pair k, per nk tile
                    for j in range(5):
                        s_step(nb, nh0, cur[k + 1], j)
                        for hh in (h0, h0 + 1):
                            av_step(b, hh, avs[hh], staP, j)
                    s_epilogue(cur[k + 1])
                else:
                    for j in range(5):
                        for hh in (h0, h0 + 1):
                            av_step(b, hh, avs[hh], staP, j)
                drs = {hh: av_evict1(avs[hh]) for hh in (h0, h0 + 1)}
                if k + 1 < 12:
                    mult_phase(cur[k + 1])
                fins = av_pass2(b, h0, avs, drs, staP)
                while fq:           # leftovers (before the omults: proj
                    fq.pop(0)()     # must not depend on this pair's o writes)
                for hh in (h0, h0 + 1):
                    omult(b, hh, fins[hh], 0)
                for hh in (h0, h0 + 1):
                    omult(b, hh, fins[hh], 1)
                del cur[k]
            # ----- remaining output projection: one DMA per outc tile ------
            for t in range(6):
                ott = oevp.tile([P, N], bf16, tag="oevt", name="oevt")
                for (f0, fsz) in PFREE1:
                    ps = ps_s.tile([P, 1024], f32, tag="ps_s", name="psmm")
                    for ki in range(6):
                        nc.tensor.matmul(
                            ps[:, 0:fsz],
                            pt[ki][:, P * t : P * (t + 1)],
                            o[ki][:, f0 : f0 + fsz],
                            start=(ki == 0),
                            stop=(ki == 5),
                        )
                    nc.scalar.activation(
                        ott[:, f0 - N : f0 - N + fsz], ps[:, 0:fsz], Act.Identity,
                        bias=pb[:, t : t + 1],
                    )
                nc.sync.dma_start(out[P * t : P * (t + 1), N:NT], ott[:, 0:N])

    nc.compile()
    return nc


def _get_nc():
    if "nc" not in _CACHE:
        _CACHE["nc"] = _build()
    return _CACHE["nc"]


def make_in_maps(x, rel_pos_bias, qkv_w, q_bias, v_bias, proj_w, proj_b):
    bf = ml_dtypes.bfloat16
    x = np.asarray(x, dtype=np.float32)
    rel_pos_bias = np.asarray(rel_pos_bias, dtype=np.float32)
    qkv_w = np.asarray(qkv_w, dtype=np.float32)
    q_bias = np.asarray(q_bias, dtype=np.float32)
    v_bias = np.asarray(v_bias, dtype=np.float32)
    proj_w = np.asarray(proj_w, dtype=np.float32)
    proj_b = np.asarray(proj_b, dtype=np.float32)

    wqkvT = np.ascontiguousarray(qkv_w.T).astype(bf)                    # [768, 2304]
    qbias = np.ascontiguousarray((q_bias * 0.125).reshape(6, P).T)      # [128, 6]
    # v_bias folds into the projection bias: attn rows sum to 1, so
    # out = attn @ (x Wv^T + v_b) Wp^T + p_b = attn x Wv^T Wp^T + (p_b + Wp v_b)
    pbias_full = proj_b + proj_w @ v_bias
    # exp of the transposed rel-pos bias: applied multiplicatively after exp(S);
    # rows padded 577->640 (5*128) so each head loads as a single strided DMA
    relbT = np.zeros((H, 640, N), dtype=bf)
    relbT[:, :N, :] = np.exp(rel_pos_bias[0].transpose(0, 2, 1)).astype(bf)
    projT = np.ascontiguousarray(proj_w.T).astype(bf)                   # [768, 768]
    pbias = np.ascontiguousarray(pbias_full.reshape(6, P).T)            # [128, 6]

    in_maps = []
    for c in range(NCORES):
        xT = np.ascontiguousarray(
            x[BPC * c : BPC * (c + 1)].reshape(NT, C).T
        ).astype(bf)                                                    # [768, 1154]
        in_maps.append(
            dict(
                xT=xT,
                wqkvT=wqkvT,
                qbias=qbias,
                relbT=relbT,
                projT=projT,
                pbias=pbias,
            )
        )
    return in_maps


def kernel(x, rel_pos_bias, qkv_w, q_bias, v_bias, proj_w, proj_b):
    from concourse import bass_utils

    in_maps = make_in_maps(x, rel_pos_bias, qkv_w, q_bias, v_bias, proj_w, proj_b)
    nc = _get_nc()
    res = bass_utils.run_bass_kernel_spmd(nc, in_maps, core_ids=list(range(NCORES)))
    outs = []
    for c in range(NCORES):
        oT = res.results[c]["out"]                                      # [768, 1154] bf16
        outs.append(np.ascontiguousarray(oT.T).astype(np.float32).reshape(BPC, N, C))
    return np.concatenate(outs, axis=0)


# revision 15
# speedup vs baseline: 1.0354x; 1.0354x over previous
"""Multi-head attention (B=16, N=577, C=768, H=12) on 8 TRN2 NeuronCores.

Strategy: pure data parallelism over batch (2 images per core, no
collectives). Per core, everything is computed "channels-on-partitions"
(transposed) so that no on-device transposes are ever needed:

  qkT[outc, tok]  = qkv_wT-tiles.T @ xT          (q scaled 1/8 + bias on evict)
  V[tok, outc]    = xT-tiles.T @ qkv_wT          (natural layout; col 64 of
                                                  each 65-block = 1; v_bias is
                                                  folded into proj bias on host
                                                  since sum_k attn[t,k] = 1)
  S^T[nk, nq]     = K^T-tiles.T @ Q^T            (K=64 contraction)
  E^T             = exp(S^T) * exp(relbT)        (host precomputes exp of the
                                                  transposed rel-pos bias; no
                                                  max subtraction -- logits are
                                                  bounded ~|7| for this problem)
  O'^T[65, nq]    = [V_h | 1]-tiles.T @ E^T      (row 64 = softmax denominator)
  O^T             = O'^T[0:64] * bcast(1/O'^T[64])   (recip + o-mult read PSUM
                                                  directly; no evict copies)
  out^T[co, tok]  = projT-tiles.T @ O^T + proj_b (+ proj_w @ v_bias)

Round-2 performance notes (see git-less history in the repo docstrings):
  - S matmuls emitted 512p0, 512p1, rump0, rump1 so the two heads' row-group
    tiles (0,0)/(64,0) run concurrently in the PE array (MMs are strict FIFO
    in issue order; pairing the same-F ones makes both overlap)
  - the normalize chain reads PSUM directly: reciprocal_approx_fast takes the
    denominator row straight out of the AV psum, the o-mult tensor_tensor
    takes O' straight out of psum (one psum operand + one sbuf operand is
    legal), so the old dr/ost copies (~28us of DVE) are gone
  - weights/x live in per-DMA-region tiles (wq0/wk0/wqr/wkr/wv, xa/xb) so
    preamble DMAs have no same-tile semaphore coupling, and the preamble is
    issued on BOTH hwdge queues (sync + scalar) -- x on sync, weights on
    scalar -- so the first matmul starts ~10us instead of ~15.4us
  - v_bias folded into proj bias on host (attn rows sum to 1), so the V
    eviction is a plain copy and vb/vbr tiles + broadcast disappear
  - fill rebalanced: qk chunks in iters 0-3 (evicts split ACT/DVE), V b1 +
    projT dma in 4-5, two b0-proj chunks in each of 6-11 (DVE evicts except
    the last iteration) so the PE never idles long enough to trip the HAM
    MID window (re-throttle to K=4/8)
  - pair-0's S steps interleave with the b0 V groups in the lead-in
  - output is written bf16 (host casts back to f32): halves the output DMA
    bytes; adds ~0.1% relative error against a 2e-2 gate
  - fp8 was evaluated and rejected: any fp8 quantization in the attention
    path contributes its full ~2-3% relative error to the output, busting
    the 2e-2 gate

Host side pre-transposes all inputs (and converts to bf16) and transposes
the output back. PSUM accumulation is f32 throughout.
"""
import numpy as np
import ml_dtypes

B, N, C, H, HD = 16, 577, 768, 12, 64
NCORES = 8
BPC = B // NCORES          # batches per core: 2
NT = BPC * N               # tokens per core: 1154
P = 128

# token-free-dim chunks over NT (matmul free dim <= 512 for f32 psum)
TFREE = [(0, 512), (512, 512), (1024, 130)]
# nk (key token) tiles over N
NKT = [(0, 128), (128, 128), (256, 128), (384, 128), (512, 65)]

_CACHE = {}

# Debug toggles for the fused normalize chain
RECIP_FROM_PSUM = False
OMULT_FROM_PSUM = True
# Emit sim-only memsets of never-read psum regions so CoreSim's
# uninitialized-read checker passes (simtest.py sets this True).
SIM_MEMSET = False


def _build():
    import concourse.tile as tile
    from concourse import bacc, mybir

    bf16 = mybir.dt.bfloat16
    f32 = mybir.dt.float32
    Alu = mybir.AluOpType
    Act = mybir.ActivationFunctionType

    nc = bacc.Bacc(
        "TRN2",
        target_bir_lowering=False,
        debug=False,
        enable_asserts=False,
        num_devices=NCORES,
    )
    xT = nc.dram_tensor("xT", [C, NT], bf16, kind="ExternalInput").ap()
    wqkvT = nc.dram_tensor("wqkvT", [C, 3 * C], bf16, kind="ExternalInput").ap()
    qbias = nc.dram_tensor("qbias", [P, 6], f32, kind="ExternalInput").ap()
    relbT = nc.dram_tensor("relbT", [H, 640, N], bf16, kind="ExternalInput").ap()
    projT = nc.dram_tensor("projT", [C, C], bf16, kind="ExternalInput").ap()
    pbias = nc.dram_tensor("pbias", [P, 6], f32, kind="ExternalInput").ap()
    out = nc.dram_tensor("out", [C, NT], bf16, kind="ExternalOutput").ap()

    with tile.TileContext(nc) as tc:
        with (
            tc.tile_pool(name="persist", bufs=1) as pp,
            tc.tile_pool(name="relb", bufs=2) as relp,
            tc.tile_pool(name="st", bufs=2) as stp,
            tc.tile_pool(name="dn", bufs=4) as dnp,
            tc.tile_pool(name="oev", bufs=3) as oevp,
            tc.tile_pool(name="pss", bufs=2, space="PSUM") as ps_s,
            tc.tile_pool(name="psrump", bufs=1, space="PSUM") as ps_r,
            tc.tile_pool(name="pso", bufs=2, space="PSUM") as ps_o,
        ):
            # ---------------- Phase A: load weights / constants ----------
            # One tile per DMA region so there is no same-tile write
            # coupling; x chunks go on the sync hwdge queue, weights +
            # biases on the scalar hwdge queue (parallel transfer rings).
            qb = pp.tile([P, 6], f32, tag="qb", name="qb")
            pb = pp.tile([P, 6], f32, tag="pb", name="pb")
            xa = pp.tile([P, 6, 512], bf16, tag="xa", name="xa")        # tok 0:512
            xb = pp.tile([P, 6, NT - 512], bf16, tag="xb", name="xb")   # tok 512:NT
            wq0 = pp.tile([P, 6, P], bf16, tag="wq0", name="wq0")       # qkv col 0:128
            wk0 = pp.tile([P, 6, P], bf16, tag="wk0", name="wk0")       # col 768:896
            wqr = pp.tile([P, 6, 5 * P], bf16, tag="wqr", name="wqr")   # col 128:768
            wkr = pp.tile([P, 6, 5 * P], bf16, tag="wkr", name="wkr")   # col 896:1536
            wv = pp.tile([P, 6, C], bf16, tag="wv", name="wv")          # col 1536:2304
            ptall = pp.tile([P, 6, C], bf16, tag="ptall", name="ptall")
            pt = [ptall[:, i, :] for i in range(6)]

            def xsl(ki, f0, fsz):
                # x slice for contraction row-block ki, tokens [f0, f0+fsz)
                if f0 + fsz <= 512:
                    return xa[:, ki, f0 : f0 + fsz]
                assert f0 >= 512
                return xb[:, ki, f0 - 512 : f0 - 512 + fsz]

            def wsl(t, ki):
                # qkv weight columns [128*t, 128*(t+1)) for row-block ki
                if t == 0:
                    return wq0[:, ki, :]
                if t == 6:
                    return wk0[:, ki, :]
                if t < 6:
                    return wqr[:, ki, P * (t - 1) : P * t]
                return wkr[:, ki, P * (t - 7) : P * (t - 6)]

            # sync queue: x (the long pole for the first matmuls)
            nc.sync.dma_start(
                xa[:, :, :], xT[:, 0:512].rearrange("(i p) n -> p i n", p=P)
            )
            nc.sync.dma_start(
                xb[:, :, :], xT[:, 512:NT].rearrange("(i p) n -> p i n", p=P)
            )
            # scalar queue: weights + biases, ordered by first use
            nc.scalar.dma_start(
                wq0[:, :, :], wqkvT[:, 0:P].rearrange("(i p) n -> p i n", p=P)
            )
            nc.scalar.dma_start(
                wk0[:, :, :], wqkvT[:, 6 * P : 7 * P].rearrange("(i p) n -> p i n", p=P)
            )
            nc.scalar.dma_start(qb[:], qbias[:])
            nc.scalar.dma_start(pb[:], pbias[:])
            nc.scalar.dma_start(
                wv[:, :, :], wqkvT[:, 2 * C : 3 * C].rearrange("(i p) n -> p i n", p=P)
            )
            nc.scalar.dma_start(
                wqr[:, :, :], wqkvT[:, P : 6 * P].rearrange("(i p) n -> p i n", p=P)
            )
            nc.scalar.dma_start(
                wkr[:, :, :],
                wqkvT[:, 7 * P : 2 * C].rearrange("(i p) n -> p i n", p=P),
            )

            # ---------------- persistent result tiles ----------------------
            # qk[t] for t in 0..11: [128, NT] bf16, outc block t (q: 0-5, k: 6-11)
            qk = []
            for t in range(12):
                qk.append(pp.tile([P, NT], bf16, tag=f"qk{t}", name=f"qk{t}"))
            # o[t]: [128, NT] bf16 -- O^T assembled for the projection
            o = []
            for t in range(6):
                o.append(pp.tile([P, NT], bf16, tag=f"o{t}", name=f"o{t}"))
            v = [[None] * 5 for _ in range(BPC)]

            def qk_chunk_emit(t, f0, fsz, eng):
                ps = ps_s.tile([P, 1024], f32, tag="ps_s", name="psmm")
                for ki in range(6):
                    nc.tensor.matmul(
                        ps[:, 0:fsz],
                        wsl(t, ki),
                        xsl(ki, f0, fsz),
                        start=(ki == 0),
                        stop=(ki == 5),
                    )
                if t < 6:  # q: scale 1/8 + bias (pre-scaled on host)
                    if eng == "act":
                        nc.scalar.activation(
                            qk[t][:, f0 : f0 + fsz], ps[:, 0:fsz], Act.Identity,
                            bias=qb[:, t : t + 1], scale=0.125,
                        )
                    else:
                        nc.vector.tensor_scalar(
                            qk[t][:, f0 : f0 + fsz], ps[:, 0:fsz], 0.125,
                            qb[:, t : t + 1], op0=Alu.mult, op1=Alu.add,
                        )
                else:  # k: plain copy (k bias is zero)
                    if eng == "act":
                        nc.scalar.copy(qk[t][:, f0 : f0 + fsz], ps[:, 0:fsz])
                    else:
                        nc.vector.tensor_copy(qk[t][:, f0 : f0 + fsz], ps[:, 0:fsz])

            def qk_group(t):
                for (f0, fsz) in TFREE:
                    qk_chunk_emit(t, f0, fsz, "act")

            def v_half_emit(bb, j, half):
                # V projection (natural layout) for batch bb, token tile j.
                # v[bb][j]: [nksz, 780] bf16, 12 head-blocks of [V_h(64) | 1]
                nk0, nksz = NKT[j]
                if half == 0:
                    v[bb][j] = pp.tile(
                        [P, 12 * 65], bf16, tag=f"v{bb}_{j}", name=f"v{bb}_{j}"
                    )
                vt = v[bb][j]
                v3 = vt[:, :].rearrange("p (h w) -> p h w", w=65)
                if half == 0:
                    nc.gpsimd.memset(v3[:, :, 64:65], 1.0)
                tok0 = bb * N + nk0
                f0 = 384 * half
                ps = ps_s.tile([P, 1024], f32, tag="ps_s", name="psmm")
                for ki in range(6):
                    nc.tensor.matmul(
                        ps[0:nksz, 0:384],
                        xsl(ki, tok0, nksz),
                        wv[:, ki, f0 : f0 + 384],
                        start=(ki == 0),
                        stop=(ki == 5),
                    )
                ps3 = ps[0:nksz, 0:384].rearrange("p (h w) -> p h w", w=64)
                nc.vector.tensor_copy(
                    v3[0:nksz, 6 * half : 6 * half + 6, 0:64], ps3[:, :, :]
                )

            def proj_group(t, f0, fsz, eng):
                ps = ps_s.tile([P, 1024], f32, tag="ps_s", name="psmm")
                for ki in range(6):
                    nc.tensor.matmul(
                        ps[:, 0:fsz],
                        pt[ki][:, P * t : P * (t + 1)],
                        o[ki][:, f0 : f0 + fsz],
                        start=(ki == 0),
                        stop=(ki == 5),
                    )
                ot = oevp.tile([P, 512], bf16, tag="oev", name="oev")
                if eng == "act":
                    nc.scalar.activation(
                        ot[:, 0:fsz], ps[:, 0:fsz], Act.Identity,
                        bias=pb[:, t : t + 1],
                    )
                else:
                    nc.vector.tensor_scalar(
                        ot[:, 0:fsz], ps[:, 0:fsz], pb[:, t : t + 1], None,
                        op0=Alu.add,
                    )
                nc.sync.dma_start(out[P * t : P * (t + 1), f0 : f0 + fsz], ot[:, 0:fsz])

            # -------------- pipelined attention phases ---------------------
            # staP/rbaP: [128, 2*5N] bf16, head parity pr at cols [pr*5N, (pr+1)*5N)
            def s_prologue(b, h0):
                # rel-bias DMAs + tile allocs for the pair (h0, h0+1)
                rbaP = relp.tile([P, 10 * N], bf16, tag="rba", name="rba")
                staP = stp.tile([P, 10 * N], bf16, tag="sta", name="sta")
                r4 = rbaP[:, :].rearrange("p (h j q) -> p h j q", h=2, q=N)
                nc.sync.dma_start(
                    r4[:, :, :, :],
                    relbT[h0 : h0 + 2, 0:640, :].rearrange(
                        "h (j p) q -> p h j q", p=P
                    ),
                )
                rump = ps_r.tile([P, 1024], f32, tag="rump", name="rump")
                if SIM_MEMSET:
                    # rows 65:128 of the j=4 rump chunks are never written by
                    # the S matmuls (nksz=65) but the strided epilogue exp
                    # reads them (the exp'd garbage is itself never read);
                    # zero them so CoreSim's uninitialized-read check passes
                    rz = rump[65:128, :].rearrange("p (h q) -> p h q", h=2)
                    nc.gpsimd.memset(rz[:, :, 260:325], 0.0)
                return staP, rbaP, rump

            def s_step(b, h0, st, j):
                # S matmuls + one pair-merged exp evict for nk tile j.
                # Emission order pairs the two 512-wide MMs (row groups
                # (0,0) / (64,0) -> concurrent), then the two rump MMs.
                staP, rbaP, rump = st
                qt = h0 // 2
                nk0, nksz = NKT[j]
                ps = ps_s.tile([P, 1024], f32, tag="ps_s", name="pss")
                lk = {}
                for hh in (h0, h0 + 1):
                    pr = hh % 2
                    qoff = pr * 64
                    lk[hh] = qk[6 + qt][
                        qoff : qoff + 64, b * N + nk0 : b * N + nk0 + nksz
                    ]
                    nc.tensor.matmul(
                        ps[0:nksz, 512 * pr : 512 * pr + 512],
                        lk[hh],
                        qk[qt][qoff : qoff + 64, b * N : b * N + 512],
                        start=True,
                        stop=True,
                    )
                for hh in (h0, h0 + 1):
                    pr = hh % 2
                    qoff = pr * 64
                    nc.tensor.matmul(
                        rump[0:nksz, 512 * pr + 65 * j : 512 * pr + 65 * j + 65],
                        lk[hh],
                        qk[qt][qoff : qoff + 64, b * N + 512 : b * N + N],
                        start=True,
                        stop=True,
                    )
                # one exp for both heads' 512-chunks (adjacent psum banks)
                s2 = staP[:, :].rearrange("p (h q) -> p h q", h=2)
                p2 = ps[:, :].rearrange("p (h q) -> p h q", h=2)
                nc.scalar.activation(
                    s2[0:nksz, :, N * j : N * j + 512],
                    p2[0:nksz, :, :],
                    Act.Exp,
                )

            def s_epilogue(st):
                # one strided exp for all ten 65-wide rumps of the pair
                # (rows 65:128 of the j=4 chunks hold garbage -- never read)
                staP, rbaP, rump = st
                s3 = staP[:, :].rearrange("p (h j q) -> p h j q", h=2, q=N)
                r3 = rump[:, :].rearrange("p (h q) -> p h q", h=2)[
                    :, :, 0:325
                ].rearrange("p h (j q) -> p h j q", q=65)
                nc.scalar.activation(
                    s3[:, :, 0:5, 512:577], r3[:, :, :, :], Act.Exp
                )

            def mult_phase(st):
                # multiplicative rel-bias, both heads at once, on DVE.
                # The j=4 block (cols 4N:5N) only has 65 valid nk rows for
                # queries 0:512 (the exp never writes rows 65:128 there), so
                # it gets its own row-restricted op.
                staP, rbaP, rump = st
                s2 = staP[:, :].rearrange("p (h q) -> p h q", h=2)
                r2 = rbaP[:, :].rearrange("p (h q) -> p h q", h=2)
                nc.vector.tensor_tensor(
                    s2[:, :, 0 : 3 * N], s2[:, :, 0 : 3 * N], r2[:, :, 0 : 3 * N],
                    op=Alu.mult,
                )
                nc.vector.tensor_tensor(
                    s2[:, :, 3 * N : 4 * N], s2[:, :, 3 * N : 4 * N],
                    r2[:, :, 3 * N : 4 * N],
                    op=Alu.mult,
                )
                nc.vector.tensor_tensor(
                    s2[0:65, :, 4 * N : 5 * N], s2[0:65, :, 4 * N : 5 * N],
                    r2[0:65, :, 4 * N : 5 * N],
                    op=Alu.mult,
                )

            def av_alloc(hh):
                # pass-1 psum: [65, 512] (one bank per head, both heads live
                # until the o-mult reads them straight out of PSUM)
                ps1 = ps_o.tile([65, 512], f32, tag="o", name="pso1")
                return ps1

            def av_step(b, hh, ps1, staP, j):
                pr = hh % 2
                nk0, nksz = NKT[j]
                lv = v[b][j][0:nksz, 65 * hh : 65 * hh + 65]
                nc.tensor.matmul(
                    ps1[0:65, 0:512],
                    lv,
                    staP[0:nksz, 5 * N * pr + N * j : 5 * N * pr + N * j + 512],
                    start=(j == 0),
                    stop=(j == 4),
                )

            def av_evict1(ps1):
                # pass-1 psum covers ALL nk for queries 0:512, so the A-half
                # denominator is final here: reciprocal of the denominator row
                # + broadcast for queries 0:512 run while pass 2 computes
                rr = dnp.tile([1, N], f32, tag="rr", name="rr")
                if RECIP_FROM_PSUM:
                    nc.vector.reciprocal_approx_fast(
                        rr[0:1, 0:512], ps1[64:65, 0:512]
                    )
                else:
                    dr = dnp.tile([1, N], f32, tag="dr", name="dr")
                    nc.vector.tensor_copy(dr[0:1, 0:512], ps1[64:65, 0:512])
                    nc.vector.reciprocal_approx_fast(rr[0:1, 0:512], dr[0:1, 0:512])
                rb = dnp.tile([64, N], f32, tag="rbb", name="rbb")
                nc.gpsimd.partition_broadcast(rb[0:64, 0:512], rr[0:1, 0:512])
                return rr, rb

            def av_pass2(b, h0, avs, drs, staP):
                # nq rump (cols 512:577) for both heads, one ps_r-pool tile
                # (bank-split h0/h1); then finish: recip from psum + broadcast
                ps2 = ps_r.tile([P, 1024], f32, tag="rump", name="pso2")
                fins = {}
                for hh in (h0, h0 + 1):
                    pr = hh % 2
                    for j, (nk0, nksz) in enumerate(NKT):
                        lv = v[b][j][0:nksz, 65 * hh : 65 * hh + 65]
                        nc.tensor.matmul(
                            ps2[0:65, 512 * pr : 512 * pr + 65],
                            lv,
                            staP[0:nksz, 5 * N * pr + N * j + 512 : 5 * N * pr + N * j + N],
                            start=(j == 0),
                            stop=(j == 4),
                        )
                for hh in (h0, h0 + 1):
                    pr = hh % 2
                    rr, rb = drs[hh]
                    if RECIP_FROM_PSUM:
                        nc.vector.reciprocal_approx_fast(
                            rr[0:1, 512:577], ps2[64:65, 512 * pr : 512 * pr + 65]
                        )
                    else:
                        dr2 = dnp.tile([1, N], f32, tag="dr2", name="dr2")
                        nc.vector.tensor_copy(
                            dr2[0:1, 512:577], ps2[64:65, 512 * pr : 512 * pr + 65]
                        )
                        nc.vector.reciprocal_approx_fast(
                            rr[0:1, 512:577], dr2[0:1, 512:577]
                        )
                    nc.gpsimd.partition_broadcast(rb[0:64, 512:577], rr[0:1, 512:577])
                    fins[hh] = (avs[hh], ps2, rb)
                return fins

            def omult(b, hh, fin, half):
                # O = O'/denominator, reading O' straight out of PSUM
                ps1, ps2, rb = fin
                pr = hh % 2
                qt = hh // 2
                qoff = (hh % 2) * 64
                if half == 0:
                    if not OMULT_FROM_PSUM:
                        ost = dnp.tile([64, 512], f32, tag="ost", name="ost")
                        nc.vector.tensor_copy(ost[0:64, 0:512], ps1[0:64, 0:512])
                        src = ost[0:64, 0:512]
                    else:
                        src = ps1[0:64, 0:512]
                    nc.vector.tensor_tensor(
                        o[qt][qoff : qoff + 64, b * N : b * N + 512],
                        src,
                        rb[0:64, 0:512],
                        op=Alu.mult,
                    )
                else:
                    if not OMULT_FROM_PSUM:
                        ost = dnp.tile([64, 512], f32, tag="ost2", name="ost2")
                        nc.vector.tensor_copy(
                            ost[0:64, 0:65], ps2[0:64, 512 * pr : 512 * pr + 65]
                        )
                        src = ost[0:64, 0:65]
                    else:
                        src = ps2[0:64, 512 * pr : 512 * pr + 65]
                    nc.vector.tensor_tensor(
                        o[qt][qoff : qoff + 64, b * N + 512 : b * N + N],
                        src,
                        rb[0:64, 512:577],
                        op=Alu.mult,
                    )

            # proj token chunks, batch-aligned
            PFREE0 = [(0, 512), (512, 65)]           # batch 0 tokens
            PFREE1 = [(577, 512), (1089, 65)]        # batch 1 tokens

            # ------------- pipelined emission schedule ---------------------
            def qk_chunk(t, ci, eng):
                def f():
                    f0, fsz = TFREE[ci]
                    qk_chunk_emit(t, f0, fsz, eng)
                return f

            def v_half(bb, j, half):
                def f():
                    v_half_emit(bb, j, half)
                return f

            def pj(t, ci, eng="dve"):
                def f():
                    f0, fsz = PFREE0[ci]
                    proj_group(t, f0, fsz, eng)
                return f

            def pt_dma():
                def f():
                    nc.sync.dma_start(
                        ptall[:, :, :], projT[:, :].rearrange("(i p) n -> p i n", p=P)
                    )
                return f

            # b1-proj partial accumulation: during iteration 11, o[0..4]'s b1
            # columns are complete (pairs 6..10), so tiles t=0,1 can run
            # their first five contraction steps early; only the ki=5 matmul
            # + evict + DMA remain for the tail. The psum tiles stay live
            # (no other ps_s user between the partials and the finish).
            pre_ps = {}

            def pj_b1_partial(t):
                def f():
                    ps = ps_s.tile([P, 1024], f32, tag="ps_s", name="psmm")
                    pre_ps[t] = ps
                    for ci, (f0, fsz) in enumerate(PFREE1):
                        for ki in range(5):
                            nc.tensor.matmul(
                                ps[:, 577 * ci : 577 * ci + fsz],
                                pt[ki][:, P * t : P * (t + 1)],
                                o[ki][:, f0 : f0 + fsz],
                                start=(ki == 0),
                                stop=False,
                            )
                return f

            fill = {
                0: [qk_chunk(2, 0, "act"), qk_chunk(8, 0, "act"),
                    qk_chunk(2, 1, "act"), qk_chunk(8, 1, "act"),
                    qk_chunk(2, 2, "act"), qk_chunk(8, 2, "act")],
                1: [qk_chunk(3, 0, "act"), qk_chunk(9, 0, "act"),
                    qk_chunk(3, 1, "act"), qk_chunk(9, 1, "act"),
                    qk_chunk(3, 2, "act"), qk_chunk(9, 2, "act")],
                2: [qk_chunk(4, 0, "act"), qk_chunk(10, 0, "act"),
                    qk_chunk(4, 1, "act"), qk_chunk(10, 1, "act"),
                    qk_chunk(4, 2, "act"), qk_chunk(10, 2, "act")],
                3: [qk_chunk(5, 0, "act"), qk_chunk(11, 0, "act"),
                    qk_chunk(5, 1, "act"), qk_chunk(11, 1, "act"),
                    qk_chunk(5, 2, "act"), qk_chunk(11, 2, "act")],
                4: [pt_dma(), v_half(1, 0, 0), v_half(1, 0, 1),
                    v_half(1, 1, 0), v_half(1, 1, 1), v_half(1, 2, 0)],
                5: [v_half(1, 2, 1), v_half(1, 3, 0), v_half(1, 3, 1),
                    v_half(1, 4, 0), v_half(1, 4, 1)],
                6: [pj(0, 0, "act"), pj(0, 1, "act")],
                7: [pj(1, 0, "act"), pj(1, 1, "act")],
                8: [pj(2, 0, "act"), pj(2, 1, "act")],
                9: [pj(3, 0, "act"), pj(3, 1, "act")],
                10: [pj(4, 0, "act"), pj(4, 1, "act")],
                11: [pj(5, 0, "act"), pj(5, 1, "act"),
                     pj_b1_partial(0), pj_b1_partial(1)],
            }

            # ---- lead-in: qk 0/6, then pair-0 S interleaved with b0 V ----
            qk_group(0)
            qk_group(6)
            pairs = [(b, h0) for b in range(BPC) for h0 in range(0, 12, 2)]
            st = s_prologue(*pairs[0])
            for j in range(5):
                s_step(*pairs[0], st, j)
                v_half_emit(0, j, 0)
                v_half_emit(0, j, 1)
            s_epilogue(st)
            qk_group(1)
            qk_group(7)
            mult_phase(st)
            cur = {0: st}
            for k in range(12):
                b, h0 = pairs[k]
                staP = cur[k][0]
                fq = list(fill[k])
                avs = {hh: av_alloc(hh) for hh in (h0, h0 + 1)}
                if k + 1 < 12:
                    nb, nh0 = pairs[k + 1]
                    cur[k + 1] = s_prologue(nb, nh0)
                    # interleave: S of pair k+1 with AV of pair k, per nk tile
                    for j in range(5):
                        s_step(nb, nh0, cur[k + 1], j)
                        for hh in (h0, h0 + 1):
                            av_step(b, hh, avs[hh], staP, j)
                    s_epilogue(cur[k + 1])
                else:
                    for j in range(5):
                        for hh in (h0, h0 + 1):
                            av_step(b, hh, avs[hh], staP, j)
                drs = {hh: av_evict1(avs[hh]) for hh in (h0, h0 + 1)}
                if k + 1 < 12:
                    mult_phase(cur[k + 1])
                fins = av_pass2(b, h0, avs, drs, staP)
                while fq:           # leftovers (before the omults: proj
                    fq.pop(0)()     # must not depend on this pair's o writes)
                for hh in (h0, h0 + 1):
                    omult(b, hh, fins[hh], 0)
                for hh in (h0, h0 + 1):
                    omult(b, hh, fins[hh], 1)
                del cur[k]
            # ----- remaining output projection: one DMA per outc tile ------
            for t in range(6):
                ott = oevp.tile([P, N], bf16, tag="oevt", name="oevt")
                if t in pre_ps:
                    # finish the pre-accumulated tiles: ki=5 only
                    ps = pre_ps[t]
                    for ci, (f0, fsz) in enumerate(PFREE1):
                        nc.tensor.matmul(
                            ps[:, 577 * ci : 577 * ci + fsz],
                            pt[5][:, P * t : P * (t + 1)],
                            o[5][:, f0 : f0 + fsz],
                            start=False,
                            stop=True,
                        )
                    for ci, (f0, fsz) in enumerate(PFREE1):
                        nc.scalar.activation(
                            ott[:, f0 - N : f0 - N + fsz],
                            ps[:, 577 * ci : 577 * ci + fsz],
                            Act.Identity,
                            bias=pb[:, t : t + 1],
                        )
                else:
                    for (f0, fsz) in PFREE1:
                        ps = ps_s.tile([P, 1024], f32, tag="ps_s", name="psmm")
                        for ki in range(6):
                            nc.tensor.matmul(
                                ps[:, 0:fsz],
                                pt[ki][:, P * t : P * (t + 1)],
                                o[ki][:, f0 : f0 + fsz],
                                start=(ki == 0),
                                stop=(ki == 5),
                            )
                        nc.scalar.activation(
                            ott[:, f0 - N : f0 - N + fsz], ps[:, 0:fsz], Act.Identity,
                            bias=pb[:, t : t + 1],
                        )
                nc.sync.dma_start(out[P * t : P * (t + 1), N:NT], ott[:, 0:N])

    nc.compile()
    return nc


def _get_nc():
    if "nc" not in _CACHE:
        _CACHE["nc"] = _build()
    return _CACHE["nc"]


def make_in_maps(x, rel_pos_bias, qkv_w, q_bias, v_bias, proj_w, proj_b):
    bf = ml_dtypes.bfloat16
    x = np.asarray(x, dtype=np.float32)
    rel_pos_bias = np.asarray(rel_pos_bias, dtype=np.float32)
    qkv_w = np.asarray(qkv_w, dtype=np.float32)
    q_bias = np.asarray(q_bias, dtype=np.float32)
    v_bias = np.asarray(v_bias, dtype=np.float32)
    proj_w = np.asarray(proj_w, dtype=np.float32)
    proj_b = np.asarray(proj_b, dtype=np.float32)

    wqkvT = np.ascontiguousarray(qkv_w.T).astype(bf)                    # [768, 2304]
    qbias = np.ascontiguousarray((q_bias * 0.125).reshape(6, P).T)      # [128, 6]
    # v_bias folds into the projection bias: attn rows sum to 1, so
    # out = attn @ (x Wv^T + v_b) Wp^T + p_b = attn x Wv^T Wp^T + (p_b + Wp v_b)
    pbias_full = proj_b + proj_w @ v_bias
    # exp of the transposed rel-pos bias: applied multiplicatively after exp(S);
    # rows padded 577->640 (5*128) so each head loads as a single strided DMA
    relbT = np.zeros((H, 640, N), dtype=bf)
    relbT[:, :N, :] = np.exp(rel_pos_bias[0].transpose(0, 2, 1)).astype(bf)
    projT = np.ascontiguousarray(proj_w.T).astype(bf)                   # [768, 768]
    pbias = np.ascontiguousarray(pbias_full.reshape(6, P).T)            # [128, 6]

    in_maps = []
    for c in range(NCORES):
        xT = np.ascontiguousarray(
            x[BPC * c : BPC * (c + 1)].reshape(NT, C).T
        ).astype(bf)                                                    # [768, 1154]
        in_maps.append(
            dict(
                xT=xT,
                wqkvT=wqkvT,
                qbias=qbias,
                relbT=relbT,
                projT=projT,
                pbias=pbias,
            )
        )
    return in_maps


def kernel(x, rel_pos_bias, qkv_w, q_bias, v_bias, proj_w, proj_b):
    from concourse import bass_utils

    in_maps = make_in_maps(x, rel_pos_bias, qkv_w, q_bias, v_bias, proj_w, proj_b)
    nc = _get_nc()
    res = bass_utils.run_bass_kernel_spmd(nc, in_maps, core_ids=list(range(NCORES)))
    outs = []
    for c in range(NCORES):
        oT = res.results[c]["out"]                                      # [768, 1154] bf16
        outs.append(np.ascontiguousarray(oT.T).astype(np.float32).reshape(BPC, N, C))
    return np.concatenate(outs, axis=0)


# revision 17
# speedup vs baseline: 1.0525x; 1.0165x over previous
"""Multi-head attention (B=16, N=577, C=768, H=12) on 8 TRN2 NeuronCores.

Strategy: pure data parallelism over batch (2 images per core, no
collectives). Per core, everything is computed "channels-on-partitions"
(transposed) so that no on-device transposes are ever needed:

  qkT[outc, tok]  = qkv_wT-tiles.T @ xT          (q scaled 1/8 + bias on evict)
  V[tok, outc]    = xT-tiles.T @ qkv_wT          (natural layout; col 64 of
                                                  each 65-block = 1; v_bias is
                                                  folded into proj bias on host
                                                  since sum_k attn[t,k] = 1)
  S^T[nk, nq]     = K^T-tiles.T @ Q^T            (K=64 contraction)
  E^T             = exp(S^T) * exp(relbT)        (host precomputes exp of the
                                                  transposed rel-pos bias; no
                                                  max subtraction -- logits are
                                                  bounded ~|7| for this problem)
  O'^T[65, nq]    = [V_h | 1]-tiles.T @ E^T      (row 64 = softmax denominator)
  O^T             = O'^T[0:64] * bcast(1/O'^T[64])   (recip + o-mult read PSUM
                                                  directly; no evict copies)
  out^T[co, tok]  = projT-tiles.T @ O^T + proj_b (+ proj_w @ v_bias)

Round-2 performance notes (see git-less history in the repo docstrings):
  - S matmuls emitted 512p0, 512p1, rump0, rump1 so the two heads' row-group
    tiles (0,0)/(64,0) run concurrently in the PE array (MMs are strict FIFO
    in issue order; pairing the same-F ones makes both overlap)
  - the normalize chain reads PSUM directly: reciprocal_approx_fast takes the
    denominator row straight out of the AV psum, the o-mult tensor_tensor
    takes O' straight out of psum (one psum operand + one sbuf operand is
    legal), so the old dr/ost copies (~28us of DVE) are gone
  - weights/x live in per-DMA-region tiles (wq0/wk0/wqr/wkr/wv, xa/xb) so
    preamble DMAs have no same-tile semaphore coupling, and the preamble is
    issued on BOTH hwdge queues (sync + scalar) -- x on sync, weights on
    scalar -- so the first matmul starts ~10us instead of ~15.4us
  - v_bias folded into proj bias on host (attn rows sum to 1), so the V
    eviction is a plain copy and vb/vbr tiles + broadcast disappear
  - fill rebalanced: qk chunks in iters 0-3 (evicts split ACT/DVE), V b1 +
    projT dma in 4-5, two b0-proj chunks in each of 6-11 (DVE evicts except
    the last iteration) so the PE never idles long enough to trip the HAM
    MID window (re-throttle to K=4/8)
  - pair-0's S steps interleave with the b0 V groups in the lead-in
  - output is written bf16 (host casts back to f32): halves the output DMA
    bytes; adds ~0.1% relative error against a 2e-2 gate
  - fp8 was evaluated and rejected: any fp8 quantization in the attention
    path contributes its full ~2-3% relative error to the output, busting
    the 2e-2 gate

Host side pre-transposes all inputs (and converts to bf16) and transposes
the output back. PSUM accumulation is f32 throughout.
"""
import numpy as np
import ml_dtypes

B, N, C, H, HD = 16, 577, 768, 12, 64
NCORES = 8
BPC = B // NCORES          # batches per core: 2
NT = BPC * N               # tokens per core: 1154
P = 128

# token-free-dim chunks over NT (matmul free dim <= 512 for f32 psum)
TFREE = [(0, 512), (512, 512), (1024, 130)]
# nk (key token) tiles over N
NKT = [(0, 128), (128, 128), (256, 128), (384, 128), (512, 65)]

_CACHE = {}

# Debug toggles for the fused normalize chain
RECIP_FROM_PSUM = False
OMULT_FROM_PSUM = True
# Emit sim-only memsets of never-read psum regions so CoreSim's
# uninitialized-read checker passes (simtest.py sets this True).
SIM_MEMSET = False


def _build():
    import concourse.tile as tile
    from concourse import bacc, mybir

    bf16 = mybir.dt.bfloat16
    f32 = mybir.dt.float32
    Alu = mybir.AluOpType
    Act = mybir.ActivationFunctionType

    nc = bacc.Bacc(
        "TRN2",
        target_bir_lowering=False,
        debug=False,
        enable_asserts=False,
        num_devices=NCORES,
    )
    xT = nc.dram_tensor("xT", [C, NT], bf16, kind="ExternalInput").ap()
    wqkvT = nc.dram_tensor("wqkvT", [C, 3 * C], bf16, kind="ExternalInput").ap()
    qbias = nc.dram_tensor("qbias", [P, 6], f32, kind="ExternalInput").ap()
    relbT = nc.dram_tensor("relbT", [H, 640, N], bf16, kind="ExternalInput").ap()
    projT = nc.dram_tensor("projT", [C, C], bf16, kind="ExternalInput").ap()
    pbias = nc.dram_tensor("pbias", [P, 6], f32, kind="ExternalInput").ap()
    out = nc.dram_tensor("out", [C, NT], bf16, kind="ExternalOutput").ap()

    with tile.TileContext(nc) as tc:
        with (
            tc.tile_pool(name="persist", bufs=1) as pp,
            tc.tile_pool(name="relb", bufs=2) as relp,
            tc.tile_pool(name="st", bufs=2) as stp,
            tc.tile_pool(name="dn", bufs=4) as dnp,
            tc.tile_pool(name="oev", bufs=3) as oevp,
            tc.tile_pool(name="pss", bufs=2, space="PSUM") as ps_s,
            tc.tile_pool(name="psrump", bufs=1, space="PSUM") as ps_r,
            tc.tile_pool(name="pso", bufs=2, space="PSUM") as ps_o,
        ):
            # ---------------- Phase A: load weights / constants ----------
            # One tile per DMA region so there is no same-tile write
            # coupling; x chunks go on the sync hwdge queue, weights +
            # biases on the scalar hwdge queue (parallel transfer rings).
            qb = pp.tile([P, 6], f32, tag="qb", name="qb")
            pb = pp.tile([P, 6], f32, tag="pb", name="pb")
            xa = pp.tile([P, 6, 512], bf16, tag="xa", name="xa")        # tok 0:512
            xb = pp.tile([P, 6, NT - 512], bf16, tag="xb", name="xb")   # tok 512:NT
            wq0 = pp.tile([P, 6, P], bf16, tag="wq0", name="wq0")       # qkv col 0:128
            wk0 = pp.tile([P, 6, P], bf16, tag="wk0", name="wk0")       # col 768:896
            wqr = pp.tile([P, 6, 5 * P], bf16, tag="wqr", name="wqr")   # col 128:768
            wkr = pp.tile([P, 6, 5 * P], bf16, tag="wkr", name="wkr")   # col 896:1536
            wv = pp.tile([P, 6, C], bf16, tag="wv", name="wv")          # col 1536:2304
            ptall = pp.tile([P, 6, C], bf16, tag="ptall", name="ptall")
            pt = [ptall[:, i, :] for i in range(6)]

            def xsl(ki, f0, fsz):
                # x slice for contraction row-block ki, tokens [f0, f0+fsz)
                if f0 + fsz <= 512:
                    return xa[:, ki, f0 : f0 + fsz]
                assert f0 >= 512
                return xb[:, ki, f0 - 512 : f0 - 512 + fsz]

            def wsl(t, ki):
                # qkv weight columns [128*t, 128*(t+1)) for row-block ki
                if t == 0:
                    return wq0[:, ki, :]
                if t == 6:
                    return wk0[:, ki, :]
                if t < 6:
                    return wqr[:, ki, P * (t - 1) : P * t]
                return wkr[:, ki, P * (t - 7) : P * (t - 6)]

            # All input loads on the sync queue, ordered by first use.
            # (The two hwdge queues do not add bandwidth -- transfers
            # serialize at ~300GB/s -- so ordering is what matters, and the
            # scalar queue's hoisted ACT_TABLE_LOAD would delay its DMAs.)
            nc.sync.dma_start(
                wq0[:, :, :], wqkvT[:, 0:P].rearrange("(i p) n -> p i n", p=P)
            )
            nc.sync.dma_start(
                xa[:, :, :], xT[:, 0:512].rearrange("(i p) n -> p i n", p=P)
            )
            nc.sync.dma_start(
                wk0[:, :, :], wqkvT[:, 6 * P : 7 * P].rearrange("(i p) n -> p i n", p=P)
            )
            nc.sync.dma_start(qb[:], qbias[:])
            nc.sync.dma_start(
                xb[:, :, :], xT[:, 512:NT].rearrange("(i p) n -> p i n", p=P)
            )
            nc.sync.dma_start(
                wv[:, :, :], wqkvT[:, 2 * C : 3 * C].rearrange("(i p) n -> p i n", p=P)
            )
            nc.sync.dma_start(
                wqr[:, :, :], wqkvT[:, P : 6 * P].rearrange("(i p) n -> p i n", p=P)
            )
            nc.sync.dma_start(
                wkr[:, :, :],
                wqkvT[:, 7 * P : 2 * C].rearrange("(i p) n -> p i n", p=P),
            )
            nc.sync.dma_start(pb[:], pbias[:])

            # ---------------- persistent result tiles ----------------------
            # qk[t] for t in 0..11: [128, NT] bf16, outc block t (q: 0-5, k: 6-11)
            qk = []
            for t in range(12):
                qk.append(pp.tile([P, NT], bf16, tag=f"qk{t}", name=f"qk{t}"))
            # o[t]: [128, NT] bf16 -- O^T assembled for the projection
            o = []
            for t in range(6):
                o.append(pp.tile([P, NT], bf16, tag=f"o{t}", name=f"o{t}"))
            v = [[None] * 5 for _ in range(BPC)]

            def qk_chunk_emit(t, f0, fsz, eng):
                ps = ps_s.tile([P, 1024], f32, tag="ps_s", name="psmm")
                for ki in range(6):
                    nc.tensor.matmul(
                        ps[:, 0:fsz],
                        wsl(t, ki),
                        xsl(ki, f0, fsz),
                        start=(ki == 0),
                        stop=(ki == 5),
                    )
                if t < 6:  # q: scale 1/8 + bias (pre-scaled on host)
                    if eng == "act":
                        nc.scalar.activation(
                            qk[t][:, f0 : f0 + fsz], ps[:, 0:fsz], Act.Identity,
                            bias=qb[:, t : t + 1], scale=0.125,
                        )
                    else:
                        nc.vector.tensor_scalar(
                            qk[t][:, f0 : f0 + fsz], ps[:, 0:fsz], 0.125,
                            qb[:, t : t + 1], op0=Alu.mult, op1=Alu.add,
                        )
                else:  # k: plain copy (k bias is zero)
                    if eng == "act":
                        nc.scalar.copy(qk[t][:, f0 : f0 + fsz], ps[:, 0:fsz])
                    else:
                        nc.vector.tensor_copy(qk[t][:, f0 : f0 + fsz], ps[:, 0:fsz])

            def qk_group(t):
                for (f0, fsz) in TFREE:
                    qk_chunk_emit(t, f0, fsz, "act")

            def v_half_emit(bb, j, half):
                # V projection (natural layout) for batch bb, token tile j.
                # v[bb][j]: [nksz, 780] bf16, 12 head-blocks of [V_h(64) | 1]
                nk0, nksz = NKT[j]
                if half == 0:
                    v[bb][j] = pp.tile(
                        [P, 12 * 65], bf16, tag=f"v{bb}_{j}", name=f"v{bb}_{j}"
                    )
                vt = v[bb][j]
                v3 = vt[:, :].rearrange("p (h w) -> p h w", w=65)
                if half == 0:
                    nc.gpsimd.memset(v3[:, :, 64:65], 1.0)
                tok0 = bb * N + nk0
                f0 = 384 * half
                ps = ps_s.tile([P, 1024], f32, tag="ps_s", name="psmm")
                for ki in range(6):
                    nc.tensor.matmul(
                        ps[0:nksz, 0:384],
                        xsl(ki, tok0, nksz),
                        wv[:, ki, f0 : f0 + 384],
                        start=(ki == 0),
                        stop=(ki == 5),
                    )
                ps3 = ps[0:nksz, 0:384].rearrange("p (h w) -> p h w", w=64)
                nc.vector.tensor_copy(
                    v3[0:nksz, 6 * half : 6 * half + 6, 0:64], ps3[:, :, :]
                )

            def proj_group(t, f0, fsz, eng):
                ps = ps_s.tile([P, 1024], f32, tag="ps_s", name="psmm")
                for ki in range(6):
                    nc.tensor.matmul(
                        ps[:, 0:fsz],
                        pt[ki][:, P * t : P * (t + 1)],
                        o[ki][:, f0 : f0 + fsz],
                        start=(ki == 0),
                        stop=(ki == 5),
                    )
                ot = oevp.tile([P, 512], bf16, tag="oev", name="oev")
                if eng == "act":
                    nc.scalar.activation(
                        ot[:, 0:fsz], ps[:, 0:fsz], Act.Identity,
                        bias=pb[:, t : t + 1],
                    )
                else:
                    nc.vector.tensor_scalar(
                        ot[:, 0:fsz], ps[:, 0:fsz], pb[:, t : t + 1], None,
                        op0=Alu.add,
                    )
                nc.sync.dma_start(out[P * t : P * (t + 1), f0 : f0 + fsz], ot[:, 0:fsz])

            # -------------- pipelined attention phases ---------------------
            # staP/rbaP: [128, 2*5N] bf16, head parity pr at cols [pr*5N, (pr+1)*5N)
            def s_prologue(b, h0):
                # rel-bias DMAs + tile allocs for the pair (h0, h0+1)
                rbaP = relp.tile([P, 10 * N], bf16, tag="rba", name="rba")
                staP = stp.tile([P, 10 * N], bf16, tag="sta", name="sta")
                r4 = rbaP[:, :].rearrange("p (h j q) -> p h j q", h=2, q=N)
                nc.sync.dma_start(
                    r4[:, :, :, :],
                    relbT[h0 : h0 + 2, 0:640, :].rearrange(
                        "h (j p) q -> p h j q", p=P
                    ),
                )
                rump = ps_r.tile([P, 1024], f32, tag="rump", name="rump")
                if SIM_MEMSET:
                    # rows 65:128 of the j=4 rump chunks are never written by
                    # the S matmuls (nksz=65) but the strided epilogue exp
                    # reads them (the exp'd garbage is itself never read);
                    # zero them so CoreSim's uninitialized-read check passes
                    rz = rump[65:128, :].rearrange("p (h q) -> p h q", h=2)
                    nc.gpsimd.memset(rz[:, :, 260:325], 0.0)
                return staP, rbaP, rump

            def s_step(b, h0, st, j):
                # S matmuls + one pair-merged exp evict for nk tile j.
                # Emission order pairs the two 512-wide MMs (row groups
                # (0,0) / (64,0) -> concurrent), then the two rump MMs.
                staP, rbaP, rump = st
                qt = h0 // 2
                nk0, nksz = NKT[j]
                ps = ps_s.tile([P, 1024], f32, tag="ps_s", name="pss")
                lk = {}
                for hh in (h0, h0 + 1):
                    pr = hh % 2
                    qoff = pr * 64
                    lk[hh] = qk[6 + qt][
                        qoff : qoff + 64, b * N + nk0 : b * N + nk0 + nksz
                    ]
                    nc.tensor.matmul(
                        ps[0:nksz, 512 * pr : 512 * pr + 512],
                        lk[hh],
                        qk[qt][qoff : qoff + 64, b * N : b * N + 512],
                        start=True,
                        stop=True,
                    )
                for hh in (h0, h0 + 1):
                    pr = hh % 2
                    qoff = pr * 64
                    nc.tensor.matmul(
                        rump[0:nksz, 512 * pr + 65 * j : 512 * pr + 65 * j + 65],
                        lk[hh],
                        qk[qt][qoff : qoff + 64, b * N + 512 : b * N + N],
                        start=True,
                        stop=True,
                    )
                # one exp for both heads' 512-chunks (adjacent psum banks)
                s2 = staP[:, :].rearrange("p (h q) -> p h q", h=2)
                p2 = ps[:, :].rearrange("p (h q) -> p h q", h=2)
                nc.scalar.activation(
                    s2[0:nksz, :, N * j : N * j + 512],
                    p2[0:nksz, :, :],
                    Act.Exp,
                )

            def s_epilogue(st):
                # one strided exp for all ten 65-wide rumps of the pair
                # (rows 65:128 of the j=4 chunks hold garbage -- never read)
                staP, rbaP, rump = st
                s3 = staP[:, :].rearrange("p (h j q) -> p h j q", h=2, q=N)
                r3 = rump[:, :].rearrange("p (h q) -> p h q", h=2)[
                    :, :, 0:325
                ].rearrange("p h (j q) -> p h j q", q=65)
                nc.scalar.activation(
                    s3[:, :, 0:5, 512:577], r3[:, :, :, :], Act.Exp
                )

            def mult_phase(st):
                # multiplicative rel-bias, both heads at once, on DVE.
                # The j=4 block (cols 4N:5N) only has 65 valid nk rows for
                # queries 0:512 (the exp never writes rows 65:128 there), so
                # it gets its own row-restricted op.
                staP, rbaP, rump = st
                s2 = staP[:, :].rearrange("p (h q) -> p h q", h=2)
                r2 = rbaP[:, :].rearrange("p (h q) -> p h q", h=2)
                nc.vector.tensor_tensor(
                    s2[:, :, 0 : 3 * N], s2[:, :, 0 : 3 * N], r2[:, :, 0 : 3 * N],
                    op=Alu.mult,
                )
                nc.vector.tensor_tensor(
                    s2[:, :, 3 * N : 4 * N], s2[:, :, 3 * N : 4 * N],
                    r2[:, :, 3 * N : 4 * N],
                    op=Alu.mult,
                )
                nc.vector.tensor_tensor(
                    s2[0:65, :, 4 * N : 5 * N], s2[0:65, :, 4 * N : 5 * N],
                    r2[0:65, :, 4 * N : 5 * N],
                    op=Alu.mult,
                )

            def av_alloc(hh):
                # pass-1 psum: [65, 512] (one bank per head, both heads live
                # until the o-mult reads them straight out of PSUM)
                ps1 = ps_o.tile([65, 512], f32, tag="o", name="pso1")
                return ps1

            def av_step(b, hh, ps1, staP, j):
                pr = hh % 2
                nk0, nksz = NKT[j]
                lv = v[b][j][0:nksz, 65 * hh : 65 * hh + 65]
                nc.tensor.matmul(
                    ps1[0:65, 0:512],
                    lv,
                    staP[0:nksz, 5 * N * pr + N * j : 5 * N * pr + N * j + 512],
                    start=(j == 0),
                    stop=(j == 4),
                )

            def av_evict1(ps1):
                # pass-1 psum covers ALL nk for queries 0:512, so the A-half
                # denominator is final here: reciprocal of the denominator row
                # + broadcast for queries 0:512 run while pass 2 computes
                rr = dnp.tile([1, N], f32, tag="rr", name="rr")
                if RECIP_FROM_PSUM:
                    nc.vector.reciprocal_approx_fast(
                        rr[0:1, 0:512], ps1[64:65, 0:512]
                    )
                else:
                    dr = dnp.tile([1, N], f32, tag="dr", name="dr")
                    nc.vector.tensor_copy(dr[0:1, 0:512], ps1[64:65, 0:512])
                    nc.vector.reciprocal_approx_fast(rr[0:1, 0:512], dr[0:1, 0:512])
                rb = dnp.tile([64, N], f32, tag="rbb", name="rbb")
                nc.gpsimd.partition_broadcast(rb[0:64, 0:512], rr[0:1, 0:512])
                return rr, rb

            def av_pass2(b, h0, avs, drs, staP):
                # nq rump (cols 512:577) for both heads, one ps_r-pool tile
                # (bank-split h0/h1); then finish: recip from psum + broadcast
                ps2 = ps_r.tile([P, 1024], f32, tag="rump", name="pso2")
                fins = {}
                for hh in (h0, h0 + 1):
                    pr = hh % 2
                    for j, (nk0, nksz) in enumerate(NKT):
                        lv = v[b][j][0:nksz, 65 * hh : 65 * hh + 65]
                        nc.tensor.matmul(
                            ps2[0:65, 512 * pr : 512 * pr + 65],
                            lv,
                            staP[0:nksz, 5 * N * pr + N * j + 512 : 5 * N * pr + N * j + N],
                            start=(j == 0),
                            stop=(j == 4),
                        )
                for hh in (h0, h0 + 1):
                    pr = hh % 2
                    rr, rb = drs[hh]
                    if RECIP_FROM_PSUM:
                        nc.vector.reciprocal_approx_fast(
                            rr[0:1, 512:577], ps2[64:65, 512 * pr : 512 * pr + 65]
                        )
                    else:
                        dr2 = dnp.tile([1, N], f32, tag="dr2", name="dr2")
                        nc.vector.tensor_copy(
                            dr2[0:1, 512:577], ps2[64:65, 512 * pr : 512 * pr + 65]
                        )
                        nc.vector.reciprocal_approx_fast(
                            rr[0:1, 512:577], dr2[0:1, 512:577]
                        )
                    nc.gpsimd.partition_broadcast(rb[0:64, 512:577], rr[0:1, 512:577])
                    fins[hh] = (avs[hh], ps2, rb)
                return fins

            def omult(b, hh, fin, half):
                # O = O'/denominator, reading O' straight out of PSUM
                ps1, ps2, rb = fin
                pr = hh % 2
                qt = hh // 2
                qoff = (hh % 2) * 64
                if half == 0:
                    if not OMULT_FROM_PSUM:
                        ost = dnp.tile([64, 512], f32, tag="ost", name="ost")
                        nc.vector.tensor_copy(ost[0:64, 0:512], ps1[0:64, 0:512])
                        src = ost[0:64, 0:512]
                    else:
                        src = ps1[0:64, 0:512]
                    nc.vector.tensor_tensor(
                        o[qt][qoff : qoff + 64, b * N : b * N + 512],
                        src,
                        rb[0:64, 0:512],
                        op=Alu.mult,
                    )
                else:
                    if not OMULT_FROM_PSUM:
                        ost = dnp.tile([64, 512], f32, tag="ost2", name="ost2")
                        nc.vector.tensor_copy(
                            ost[0:64, 0:65], ps2[0:64, 512 * pr : 512 * pr + 65]
                        )
                        src = ost[0:64, 0:65]
                    else:
                        src = ps2[0:64, 512 * pr : 512 * pr + 65]
                    nc.vector.tensor_tensor(
                        o[qt][qoff : qoff + 64, b * N + 512 : b * N + N],
                        src,
                        rb[0:64, 512:577],
                        op=Alu.mult,
                    )

            # proj token chunks, batch-aligned
            PFREE0 = [(0, 512), (512, 65)]           # batch 0 tokens
            PFREE1 = [(577, 512), (1089, 65)]        # batch 1 tokens

            # ------------- pipelined emission schedule ---------------------
            def qk_chunk(t, ci, eng):
                def f():
                    f0, fsz = TFREE[ci]
                    qk_chunk_emit(t, f0, fsz, eng)
                return f

            def v_half(bb, j, half):
                def f():
                    v_half_emit(bb, j, half)
                return f

            def pj(t, ci, eng="dve"):
                def f():
                    f0, fsz = PFREE0[ci]
                    proj_group(t, f0, fsz, eng)
                return f

            def pt_dma():
                def f():
                    nc.sync.dma_start(
                        ptall[:, :, :], projT[:, :].rearrange("(i p) n -> p i n", p=P)
                    )
                return f

            # b1-proj partial accumulation: during iteration 11, o[0..4]'s b1
            # columns are complete (pairs 6..10), so tiles t=0,1 can run
            # their first five contraction steps early; only the ki=5 matmul
            # + evict + DMA remain for the tail. The psum tiles stay live
            # (no other ps_s user between the partials and the finish).
            pre_ps = {}

            def pj_b1_partial(t):
                def f():
                    ps = ps_s.tile([P, 1024], f32, tag="ps_s", name="psmm")
                    pre_ps[t] = ps
                    for ci, (f0, fsz) in enumerate(PFREE1):
                        for ki in range(5):
                            nc.tensor.matmul(
                                ps[:, 577 * ci : 577 * ci + fsz],
                                pt[ki][:, P * t : P * (t + 1)],
                                o[ki][:, f0 : f0 + fsz],
                                start=(ki == 0),
                                stop=False,
                            )
                return f

            fill = {
                0: [qk_chunk(2, 0, "act"), qk_chunk(8, 0, "act"),
                    qk_chunk(2, 1, "act"), qk_chunk(8, 1, "act"),
                    qk_chunk(2, 2, "act"), qk_chunk(8, 2, "act")],
                1: [qk_chunk(3, 0, "act"), qk_chunk(9, 0, "act"),
                    qk_chunk(3, 1, "act"), qk_chunk(9, 1, "act"),
                    qk_chunk(3, 2, "act"), qk_chunk(9, 2, "act")],
                2: [qk_chunk(4, 0, "act"), qk_chunk(10, 0, "act"),
                    qk_chunk(4, 1, "act"), qk_chunk(10, 1, "act"),
                    qk_chunk(4, 2, "act"), qk_chunk(10, 2, "act")],
                3: [qk_chunk(5, 0, "act"), qk_chunk(11, 0, "act"),
                    qk_chunk(5, 1, "act"), qk_chunk(11, 1, "act"),
                    qk_chunk(5, 2, "act"), qk_chunk(11, 2, "act")],
                4: [pt_dma(), v_half(1, 0, 0), v_half(1, 0, 1),
                    v_half(1, 1, 0), v_half(1, 1, 1), v_half(1, 2, 0)],
                5: [v_half(1, 2, 1), v_half(1, 3, 0), v_half(1, 3, 1),
                    v_half(1, 4, 0), v_half(1, 4, 1)],
                6: [pj(0, 0, "act"), pj(0, 1, "act")],
                7: [pj(1, 0, "act"), pj(1, 1, "act")],
                8: [pj(2, 0, "act"), pj(2, 1, "act")],
                9: [pj(3, 0, "act"), pj(3, 1, "act")],
                10: [pj(4, 0, "act"), pj(4, 1, "act")],
                11: [pj(5, 0, "act"), pj(5, 1, "act"),
                     pj_b1_partial(0), pj_b1_partial(1)],
            }

            # ---- lead-in: qk 0/6, then pair-0 S interleaved with b0 V ----
            qk_group(0)
            qk_group(6)
            pairs = [(b, h0) for b in range(BPC) for h0 in range(0, 12, 2)]
            st = s_prologue(*pairs[0])
            for j in range(5):
                s_step(*pairs[0], st, j)
                v_half_emit(0, j, 0)
                v_half_emit(0, j, 1)
            s_epilogue(st)
            qk_group(1)
            qk_group(7)
            mult_phase(st)
            cur = {0: st}
            for k in range(12):
                b, h0 = pairs[k]
                staP = cur[k][0]
                fq = list(fill[k])
                avs = {hh: av_alloc(hh) for hh in (h0, h0 + 1)}
                if k + 1 < 12:
                    nb, nh0 = pairs[k + 1]
                    cur[k + 1] = s_prologue(nb, nh0)
                    # interleave: S of pair k+1 with AV of pair k, per nk tile
                    for j in range(5):
                        s_step(nb, nh0, cur[k + 1], j)
                        for hh in (h0, h0 + 1):
                            av_step(b, hh, avs[hh], staP, j)
                    s_epilogue(cur[k + 1])
                else:
                    for j in range(5):
                        for hh in (h0, h0 + 1):
                            av_step(b, hh, avs[hh], staP, j)
                # mult of pair k+1 goes FIRST on the DVE queue: it gates the
                # next iteration's AV matmuls, while the denominator chain
                # below only gates this pair's o-mults
                if k + 1 < 12:
                    mult_phase(cur[k + 1])
                drs = {hh: av_evict1(avs[hh]) for hh in (h0, h0 + 1)}
                fins = av_pass2(b, h0, avs, drs, staP)
                while fq:           # leftovers (before the omults: proj
                    fq.pop(0)()     # must not depend on this pair's o writes)
                for hh in (h0, h0 + 1):
                    omult(b, hh, fins[hh], 0)
                for hh in (h0, h0 + 1):
                    omult(b, hh, fins[hh], 1)
                del cur[k]
            # ----- remaining output projection: one DMA per outc tile ------
            for t in range(6):
                ott = oevp.tile([P, N], bf16, tag="oevt", name="oevt")
                if t in pre_ps:
                    # finish the pre-accumulated tiles: ki=5 only
                    ps = pre_ps[t]
                    for ci, (f0, fsz) in enumerate(PFREE1):
                        nc.tensor.matmul(
                            ps[:, 577 * ci : 577 * ci + fsz],
                            pt[5][:, P * t : P * (t + 1)],
                            o[5][:, f0 : f0 + fsz],
                            start=False,
                            stop=True,
                        )
                    for ci, (f0, fsz) in enumerate(PFREE1):
                        nc.scalar.activation(
                            ott[:, f0 - N : f0 - N + fsz],
                            ps[:, 577 * ci : 577 * ci + fsz],
                            Act.Identity,
                            bias=pb[:, t : t + 1],
                        )
                else:
                    for (f0, fsz) in PFREE1:
                        ps = ps_s.tile([P, 1024], f32, tag="ps_s", name="psmm")
                        for ki in range(6):
                            nc.tensor.matmul(
                                ps[:, 0:fsz],
                                pt[ki][:, P * t : P * (t + 1)],
                                o[ki][:, f0 : f0 + fsz],
                                start=(ki == 0),
                                stop=(ki == 5),
                            )
                        nc.scalar.activation(
                            ott[:, f0 - N : f0 - N + fsz], ps[:, 0:fsz], Act.Identity,
                            bias=pb[:, t : t + 1],
                        )
                nc.sync.dma_start(out[P * t : P * (t + 1), N:NT], ott[:, 0:N])

    nc.compile()
    return nc


def _get_nc():
    if "nc" not in _CACHE:
        _CACHE["nc"] = _build()
    return _CACHE["nc"]


def make_in_maps(x, rel_pos_bias, qkv_w, q_bias, v_bias, proj_w, proj_b):
    bf = ml_dtypes.bfloat16
    x = np.asarray(x, dtype=np.float32)
    rel_pos_bias = np.asarray(rel_pos_bias, dtype=np.float32)
    qkv_w = np.asarray(qkv_w, dtype=np.float32)
    q_bias = np.asarray(q_bias, dtype=np.float32)
    v_bias = np.asarray(v_bias, dtype=np.float32)
    proj_w = np.asarray(proj_w, dtype=np.float32)
    proj_b = np.asarray(proj_b, dtype=np.float32)

    wqkvT = np.ascontiguousarray(qkv_w.T).astype(bf)                    # [768, 2304]
    qbias = np.ascontiguousarray((q_bias * 0.125).reshape(6, P).T)      # [128, 6]
    # v_bias folds into the projection bias: attn rows sum to 1, so
    # out = attn @ (x Wv^T + v_b) Wp^T + p_b = attn x Wv^T Wp^T + (p_b + Wp v_b)
    pbias_full = proj_b + proj_w @ v_bias
    # exp of the transposed rel-pos bias: applied multiplicatively after exp(S);
    # rows padded 577->640 (5*128) so each head loads as a single strided DMA
    relbT = np.zeros((H, 640, N), dtype=bf)
    relbT[:, :N, :] = np.exp(rel_pos_bias[0].transpose(0, 2, 1)).astype(bf)
    projT = np.ascontiguousarray(proj_w.T).astype(bf)                   # [768, 768]
    pbias = np.ascontiguousarray(pbias_full.reshape(6, P).T)            # [128, 6]

    in_maps = []
    for c in range(NCORES):
        xT = np.ascontiguousarray(
            x[BPC * c : BPC * (c + 1)].reshape(NT, C).T
        ).astype(bf)                                                    # [768, 1154]
        in_maps.append(
            dict(
                xT=xT,
                wqkvT=wqkvT,
                qbias=qbias,
                relbT=relbT,
                projT=projT,
                pbias=pbias,
            )
        )
    return in_maps


def kernel(x, rel_pos_bias, qkv_w, q_bias, v_bias, proj_w, proj_b):
    from concourse import bass_utils

    in_maps = make_in_maps(x, rel_pos_bias, qkv_w, q_bias, v_bias, proj_w, proj_b)
    nc = _get_nc()
    res = bass_utils.run_bass_kernel_spmd(nc, in_maps, core_ids=list(range(NCORES)))
    outs = []
    for c in range(NCORES):
        oT = res.results[c]["out"]                                      # [768, 1154] bf16
        outs.append(np.ascontiguousarray(oT.T).astype(np.float32).reshape(BPC, N, C))
    return np.concatenate(outs, axis=0)


# revision 22
# speedup vs baseline: 1.0580x; 1.0052x over previous
"""Multi-head attention (B=16, N=577, C=768, H=12) on 8 TRN2 NeuronCores.

Strategy: pure data parallelism over batch (2 images per core, no
collectives). Per core, everything is computed "channels-on-partitions"
(transposed) so that no on-device transposes are ever needed:

  qkT[outc, tok]  = qkv_wT-tiles.T @ xT          (q scaled 1/8 + bias on evict)
  V[tok, outc]    = xT-tiles.T @ qkv_wT          (natural layout; col 64 of
                                                  each 65-block = 1; v_bias is
                                                  folded into proj bias on host
                                                  since sum_k attn[t,k] = 1)
  S^T[nk, nq]     = K^T-tiles.T @ Q^T            (K=64 contraction)
  E^T             = exp(S^T) * exp(relbT)        (host precomputes exp of the
                                                  transposed rel-pos bias; no
                                                  max subtraction -- logits are
                                                  bounded ~|7| for this problem)
  O'^T[65, nq]    = [V_h | 1]-tiles.T @ E^T      (row 64 = softmax denominator)
  O^T             = O'^T[0:64] * bcast(1/O'^T[64])   (recip + o-mult read PSUM
                                                  directly; no evict copies)
  out^T[co, tok]  = projT-tiles.T @ O^T + proj_b (+ proj_w @ v_bias)

Round-2 performance notes (see git-less history in the repo docstrings):
  - S matmuls emitted 512p0, 512p1, rump0, rump1 so the two heads' row-group
    tiles (0,0)/(64,0) run concurrently in the PE array (MMs are strict FIFO
    in issue order; pairing the same-F ones makes both overlap)
  - the normalize chain reads PSUM directly: reciprocal_approx_fast takes the
    denominator row straight out of the AV psum, the o-mult tensor_tensor
    takes O' straight out of psum (one psum operand + one sbuf operand is
    legal), so the old dr/ost copies (~28us of DVE) are gone
  - weights/x live in per-DMA-region tiles (wq0/wk0/wqr/wkr/wv, xa/xb) so
    preamble DMAs have no same-tile semaphore coupling, and the preamble is
    issued on BOTH hwdge queues (sync + scalar) -- x on sync, weights on
    scalar -- so the first matmul starts ~10us instead of ~15.4us
  - v_bias folded into proj bias on host (attn rows sum to 1), so the V
    eviction is a plain copy and vb/vbr tiles + broadcast disappear
  - fill rebalanced: qk chunks in iters 0-3 (evicts split ACT/DVE), V b1 +
    projT dma in 4-5, two b0-proj chunks in each of 6-11 (DVE evicts except
    the last iteration) so the PE never idles long enough to trip the HAM
    MID window (re-throttle to K=4/8)
  - pair-0's S steps interleave with the b0 V groups in the lead-in
  - output is written bf16 (host casts back to f32): halves the output DMA
    bytes; adds ~0.1% relative error against a 2e-2 gate
  - fp8 was evaluated and rejected: any fp8 quantization in the attention
    path contributes its full ~2-3% relative error to the output, busting
    the 2e-2 gate

Host side pre-transposes all inputs (and converts to bf16) and transposes
the output back. PSUM accumulation is f32 throughout.
"""
import numpy as np
import ml_dtypes

B, N, C, H, HD = 16, 577, 768, 12, 64
NCORES = 8
BPC = B // NCORES          # batches per core: 2
NT = BPC * N               # tokens per core: 1154
P = 128

# token-free-dim chunks over NT (matmul free dim <= 512 for f32 psum)
TFREE = [(0, 512), (512, 512), (1024, 130)]
# nk (key token) tiles over N
NKT = [(0, 128), (128, 128), (256, 128), (384, 128), (512, 65)]

_CACHE = {}

# Debug toggles for the fused normalize chain
RECIP_FROM_PSUM = False
OMULT_FROM_PSUM = True
# Emit sim-only memsets of never-read psum regions so CoreSim's
# uninitialized-read checker passes (simtest.py sets this True).
SIM_MEMSET = False


def _build():
    import concourse.tile as tile
    from concourse import bacc, mybir

    bf16 = mybir.dt.bfloat16
    f32 = mybir.dt.float32
    Alu = mybir.AluOpType
    Act = mybir.ActivationFunctionType

    nc = bacc.Bacc(
        "TRN2",
        target_bir_lowering=False,
        debug=False,
        enable_asserts=False,
        num_devices=NCORES,
    )
    # x and the five weight-column blocks ship as separate contiguous
    # arrays so every preamble DMA reads dense DRAM at full bandwidth
    # (column-slices of one big array only reach ~150GB/s)
    xTa = nc.dram_tensor("xTa", [C, 512], bf16, kind="ExternalInput").ap()
    xTb = nc.dram_tensor("xTb", [C, NT - 512], bf16, kind="ExternalInput").ap()
    wq0d = nc.dram_tensor("wq0d", [C, P], bf16, kind="ExternalInput").ap()
    wk0d = nc.dram_tensor("wk0d", [C, P], bf16, kind="ExternalInput").ap()
    wqrd = nc.dram_tensor("wqrd", [C, 5 * P], bf16, kind="ExternalInput").ap()
    wkrd = nc.dram_tensor("wkrd", [C, 5 * P], bf16, kind="ExternalInput").ap()
    wvd = nc.dram_tensor("wvd", [C, C], bf16, kind="ExternalInput").ap()
    qbias = nc.dram_tensor("qbias", [P, 6], f32, kind="ExternalInput").ap()
    relbT = nc.dram_tensor("relbT", [H, 640, N], bf16, kind="ExternalInput").ap()
    projT = nc.dram_tensor("projT", [C, C], bf16, kind="ExternalInput").ap()
    pbias = nc.dram_tensor("pbias", [P, 6], f32, kind="ExternalInput").ap()
    out = nc.dram_tensor("out", [C, NT], bf16, kind="ExternalOutput").ap()

    with tile.TileContext(nc) as tc:
        with (
            tc.tile_pool(name="persist", bufs=1) as pp,
            tc.tile_pool(name="relb", bufs=2) as relp,
            tc.tile_pool(name="st", bufs=2) as stp,
            tc.tile_pool(name="dn", bufs=4) as dnp,
            tc.tile_pool(name="oev", bufs=3) as oevp,
            tc.tile_pool(name="pss", bufs=2, space="PSUM") as ps_s,
            tc.tile_pool(name="psrump", bufs=1, space="PSUM") as ps_r,
            tc.tile_pool(name="pso", bufs=2, space="PSUM") as ps_o,
        ):
            # ---------------- Phase A: load weights / constants ----------
            # One tile per DMA region so there is no same-tile write
            # coupling; x chunks go on the sync hwdge queue, weights +
            # biases on the scalar hwdge queue (parallel transfer rings).
            qb = pp.tile([P, 6], f32, tag="qb", name="qb")
            pb = pp.tile([P, 6], f32, tag="pb", name="pb")
            xa = pp.tile([P, 6, 512], bf16, tag="xa", name="xa")        # tok 0:512
            xb = pp.tile([P, 6, NT - 512], bf16, tag="xb", name="xb")   # tok 512:NT
            wq0 = pp.tile([P, 6, P], bf16, tag="wq0", name="wq0")       # qkv col 0:128
            wk0 = pp.tile([P, 6, P], bf16, tag="wk0", name="wk0")       # col 768:896
            wqr = pp.tile([P, 6, 5 * P], bf16, tag="wqr", name="wqr")   # col 128:768
            wkr = pp.tile([P, 6, 5 * P], bf16, tag="wkr", name="wkr")   # col 896:1536
            wv = pp.tile([P, 6, C], bf16, tag="wv", name="wv")          # col 1536:2304
            ptall = pp.tile([P, 6, C], bf16, tag="ptall", name="ptall")
            pt = [ptall[:, i, :] for i in range(6)]

            def xsl(ki, f0, fsz):
                # x slice for contraction row-block ki, tokens [f0, f0+fsz)
                if f0 + fsz <= 512:
                    return xa[:, ki, f0 : f0 + fsz]
                assert f0 >= 512
                return xb[:, ki, f0 - 512 : f0 - 512 + fsz]

            def wsl(t, ki):
                # qkv weight columns [128*t, 128*(t+1)) for row-block ki
                if t == 0:
                    return wq0[:, ki, :]
                if t == 6:
                    return wk0[:, ki, :]
                if t < 6:
                    return wqr[:, ki, P * (t - 1) : P * t]
                return wkr[:, ki, P * (t - 7) : P * (t - 6)]

            # All input loads on the sync queue, ordered by first use.
            # (The two hwdge queues do not add bandwidth -- transfers
            # serialize -- so ordering is what matters, and the scalar
            # queue's hoisted ACT_TABLE_LOAD would delay its DMAs.)
            nc.sync.dma_start(
                wq0[:, :, :], wq0d[:, :].rearrange("(i p) n -> p i n", p=P)
            )
            nc.sync.dma_start(
                xa[:, :, :], xTa[:, :].rearrange("(i p) n -> p i n", p=P)
            )
            nc.sync.dma_start(
                wk0[:, :, :], wk0d[:, :].rearrange("(i p) n -> p i n", p=P)
            )
            nc.sync.dma_start(qb[:], qbias[:])
            nc.sync.dma_start(
                xb[:, :, :], xTb[:, :].rearrange("(i p) n -> p i n", p=P)
            )
            nc.sync.dma_start(
                wv[:, :, :], wvd[:, :].rearrange("(i p) n -> p i n", p=P)
            )
            nc.sync.dma_start(
                wqr[:, :, :], wqrd[:, :].rearrange("(i p) n -> p i n", p=P)
            )
            nc.sync.dma_start(
                wkr[:, :, :], wkrd[:, :].rearrange("(i p) n -> p i n", p=P)
            )
            nc.sync.dma_start(pb[:], pbias[:])

            # ---------------- persistent result tiles ----------------------
            # qk[t] for t in 0..11: [128, NT] bf16, outc block t (q: 0-5, k: 6-11)
            qk = []
            for t in range(12):
                qk.append(pp.tile([P, NT], bf16, tag=f"qk{t}", name=f"qk{t}"))
            # o[t]: [128, NT] bf16 -- O^T assembled for the projection
            o = []
            for t in range(6):
                o.append(pp.tile([P, NT], bf16, tag=f"o{t}", name=f"o{t}"))
            v = [[None] * 5 for _ in range(BPC)]

            def qk_chunk_emit(t, f0, fsz, eng):
                ps = ps_s.tile([P, 1024], f32, tag="ps_s", name="psmm")
                for ki in range(6):
                    nc.tensor.matmul(
                        ps[:, 0:fsz],
                        wsl(t, ki),
                        xsl(ki, f0, fsz),
                        start=(ki == 0),
                        stop=(ki == 5),
                    )
                if t < 6:  # q: scale 1/8 + bias (pre-scaled on host)
                    if eng == "act":
                        nc.scalar.activation(
                            qk[t][:, f0 : f0 + fsz], ps[:, 0:fsz], Act.Identity,
                            bias=qb[:, t : t + 1], scale=0.125,
                        )
                    else:
                        nc.vector.tensor_scalar(
                            qk[t][:, f0 : f0 + fsz], ps[:, 0:fsz], 0.125,
                            qb[:, t : t + 1], op0=Alu.mult, op1=Alu.add,
                        )
                else:  # k: plain copy (k bias is zero)
                    if eng == "act":
                        nc.scalar.copy(qk[t][:, f0 : f0 + fsz], ps[:, 0:fsz])
                    else:
                        nc.vector.tensor_copy(qk[t][:, f0 : f0 + fsz], ps[:, 0:fsz])

            def qk_group(t):
                for (f0, fsz) in TFREE:
                    qk_chunk_emit(t, f0, fsz, "act")

            def v_half_emit(bb, j, half):
                # V projection (natural layout) for batch bb, token tile j.
                # v[bb][j]: [nksz, 780] bf16, 12 head-blocks of [V_h(64) | 1]
                nk0, nksz = NKT[j]
                if half == 0:
                    v[bb][j] = pp.tile(
                        [P, 12 * 65], bf16, tag=f"v{bb}_{j}", name=f"v{bb}_{j}"
                    )
                vt = v[bb][j]
                v3 = vt[:, :].rearrange("p (h w) -> p h w", w=65)
                if half == 0:
                    nc.gpsimd.memset(v3[:, :, 64:65], 1.0)
                tok0 = bb * N + nk0
                f0 = 384 * half
                ps = ps_s.tile([P, 1024], f32, tag="ps_s", name="psmm")
                for ki in range(6):
                    nc.tensor.matmul(
                        ps[0:nksz, 0:384],
                        xsl(ki, tok0, nksz),
                        wv[:, ki, f0 : f0 + 384],
                        start=(ki == 0),
                        stop=(ki == 5),
                    )
                ps3 = ps[0:nksz, 0:384].rearrange("p (h w) -> p h w", w=64)
                nc.vector.tensor_copy(
                    v3[0:nksz, 6 * half : 6 * half + 6, 0:64], ps3[:, :, :]
                )

            def proj_group(t, f0, fsz, eng):
                ps = ps_s.tile([P, 1024], f32, tag="ps_s", name="psmm")
                for ki in range(6):
                    nc.tensor.matmul(
                        ps[:, 0:fsz],
                        pt[ki][:, P * t : P * (t + 1)],
                        o[ki][:, f0 : f0 + fsz],
                        start=(ki == 0),
                        stop=(ki == 5),
                    )
                ot = oevp.tile([P, 512], bf16, tag="oev", name="oev")
                if eng == "act":
                    nc.scalar.activation(
                        ot[:, 0:fsz], ps[:, 0:fsz], Act.Identity,
                        bias=pb[:, t : t + 1],
                    )
                else:
                    nc.vector.tensor_scalar(
                        ot[:, 0:fsz], ps[:, 0:fsz], pb[:, t : t + 1], None,
                        op0=Alu.add,
                    )
                nc.sync.dma_start(out[P * t : P * (t + 1), f0 : f0 + fsz], ot[:, 0:fsz])

            # -------------- pipelined attention phases ---------------------
            # staP/rbaP: [128, 2*5N] bf16, head parity pr at cols [pr*5N, (pr+1)*5N)
            def s_prologue(b, h0):
                # rel-bias DMAs + tile allocs for the pair (h0, h0+1)
                rbaP = relp.tile([P, 10 * N], bf16, tag="rba", name="rba")
                staP = stp.tile([P, 10 * N], bf16, tag="sta", name="sta")
                r4 = rbaP[:, :].rearrange("p (h j q) -> p h j q", h=2, q=N)
                nc.sync.dma_start(
                    r4[:, :, :, :],
                    relbT[h0 : h0 + 2, 0:640, :].rearrange(
                        "h (j p) q -> p h j q", p=P
                    ),
                )
                rump = ps_r.tile([P, 1024], f32, tag="rump", name="rump")
                if SIM_MEMSET:
                    # rows 65:128 of the j=4 rump chunks are never written by
                    # the S matmuls (nksz=65) but the strided epilogue exp
                    # reads them (the exp'd garbage is itself never read);
                    # zero them so CoreSim's uninitialized-read check passes
                    rz = rump[65:128, :].rearrange("p (h q) -> p h q", h=2)
                    nc.gpsimd.memset(rz[:, :, 260:325], 0.0)
                return staP, rbaP, rump

            def s_step(b, h0, st, j):
                # S matmuls + one pair-merged exp evict for nk tile j.
                # Emission order pairs the two 512-wide MMs (row groups
                # (0,0) / (64,0) -> concurrent), then the two rump MMs.
                staP, rbaP, rump = st
                qt = h0 // 2
                nk0, nksz = NKT[j]
                ps = ps_s.tile([P, 1024], f32, tag="ps_s", name="pss")
                lk = {}
                for hh in (h0, h0 + 1):
                    pr = hh % 2
                    qoff = pr * 64
                    lk[hh] = qk[6 + qt][
                        qoff : qoff + 64, b * N + nk0 : b * N + nk0 + nksz
                    ]
                    nc.tensor.matmul(
                        ps[0:nksz, 512 * pr : 512 * pr + 512],
                        lk[hh],
                        qk[qt][qoff : qoff + 64, b * N : b * N + 512],
                        start=True,
                        stop=True,
                    )
                for hh in (h0, h0 + 1):
                    pr = hh % 2
                    qoff = pr * 64
                    nc.tensor.matmul(
                        rump[0:nksz, 512 * pr + 65 * j : 512 * pr + 65 * j + 65],
                        lk[hh],
                        qk[qt][qoff : qoff + 64, b * N + 512 : b * N + N],
                        start=True,
                        stop=True,
                    )
                # one exp for both heads' 512-chunks (adjacent psum banks)
                s2 = staP[:, :].rearrange("p (h q) -> p h q", h=2)
                p2 = ps[:, :].rearrange("p (h q) -> p h q", h=2)
                nc.scalar.activation(
                    s2[0:nksz, :, N * j : N * j + 512],
                    p2[0:nksz, :, :],
                    Act.Exp,
                )

            def s_epilogue(st):
                # one strided exp for all ten 65-wide rumps of the pair
                # (rows 65:128 of the j=4 chunks hold garbage -- never read)
                staP, rbaP, rump = st
                s3 = staP[:, :].rearrange("p (h j q) -> p h j q", h=2, q=N)
                r3 = rump[:, :].rearrange("p (h q) -> p h q", h=2)[
                    :, :, 0:325
                ].rearrange("p h (j q) -> p h j q", q=65)
                nc.scalar.activation(
                    s3[:, :, 0:5, 512:577], r3[:, :, :, :], Act.Exp
                )

            def mult_phase(st):
                # multiplicative rel-bias, both heads at once, on DVE.
                # The j=4 block (cols 4N:5N) only has 65 valid nk rows for
                # queries 0:512 (the exp never writes rows 65:128 there), so
                # it gets its own row-restricted op.
                staP, rbaP, rump = st
                s2 = staP[:, :].rearrange("p (h q) -> p h q", h=2)
                r2 = rbaP[:, :].rearrange("p (h q) -> p h q", h=2)
                nc.vector.tensor_tensor(
                    s2[:, :, 0 : 3 * N], s2[:, :, 0 : 3 * N], r2[:, :, 0 : 3 * N],
                    op=Alu.mult,
                )
                nc.vector.tensor_tensor(
                    s2[:, :, 3 * N : 4 * N], s2[:, :, 3 * N : 4 * N],
                    r2[:, :, 3 * N : 4 * N],
                    op=Alu.mult,
                )
                nc.vector.tensor_tensor(
                    s2[0:65, :, 4 * N : 5 * N], s2[0:65, :, 4 * N : 5 * N],
                    r2[0:65, :, 4 * N : 5 * N],
                    op=Alu.mult,
                )

            def av_alloc(hh):
                # pass-1 psum: [65, 512] (one bank per head, both heads live
                # until the o-mult reads them straight out of PSUM)
                ps1 = ps_o.tile([65, 512], f32, tag="o", name="pso1")
                return ps1

            def av_step(b, hh, ps1, staP, j):
                pr = hh % 2
                nk0, nksz = NKT[j]
                lv = v[b][j][0:nksz, 65 * hh : 65 * hh + 65]
                nc.tensor.matmul(
                    ps1[0:65, 0:512],
                    lv,
                    staP[0:nksz, 5 * N * pr + N * j : 5 * N * pr + N * j + 512],
                    start=(j == 0),
                    stop=(j == 4),
                )

            def av_evict1(ps1):
                # pass-1 psum covers ALL nk for queries 0:512, so the A-half
                # denominator is final here: reciprocal of the denominator row
                # + broadcast for queries 0:512 run while pass 2 computes
                rr = dnp.tile([1, N], f32, tag="rr", name="rr")
                if RECIP_FROM_PSUM:
                    nc.vector.reciprocal_approx_fast(
                        rr[0:1, 0:512], ps1[64:65, 0:512]
                    )
                else:
                    dr = dnp.tile([1, N], f32, tag="dr", name="dr")
                    nc.vector.tensor_copy(dr[0:1, 0:512], ps1[64:65, 0:512])
                    nc.vector.reciprocal_approx_fast(rr[0:1, 0:512], dr[0:1, 0:512])
                rb = dnp.tile([64, N], f32, tag="rbb", name="rbb")
                nc.gpsimd.partition_broadcast(rb[0:64, 0:512], rr[0:1, 0:512])
                return rr, rb

            def av_pass2(b, h0, avs, drs, staP):
                # nq rump (cols 512:577) for both heads, one ps_r-pool tile
                # (bank-split h0/h1); then finish: recip from psum + broadcast
                ps2 = ps_r.tile([P, 1024], f32, tag="rump", name="pso2")
                fins = {}
                for hh in (h0, h0 + 1):
                    pr = hh % 2
                    for j, (nk0, nksz) in enumerate(NKT):
                        lv = v[b][j][0:nksz, 65 * hh : 65 * hh + 65]
                        nc.tensor.matmul(
                            ps2[0:65, 512 * pr : 512 * pr + 65],
                            lv,
                            staP[0:nksz, 5 * N * pr + N * j + 512 : 5 * N * pr + N * j + N],
                            start=(j == 0),
                            stop=(j == 4),
                        )
                for hh in (h0, h0 + 1):
                    pr = hh % 2
                    rr, rb = drs[hh]
                    if RECIP_FROM_PSUM:
                        nc.vector.reciprocal_approx_fast(
                            rr[0:1, 512:577], ps2[64:65, 512 * pr : 512 * pr + 65]
                        )
                    else:
                        dr2 = dnp.tile([1, N], f32, tag="dr2", name="dr2")
                        nc.vector.tensor_copy(
                            dr2[0:1, 512:577], ps2[64:65, 512 * pr : 512 * pr + 65]
                        )
                        nc.vector.reciprocal_approx_fast(
                            rr[0:1, 512:577], dr2[0:1, 512:577]
                        )
                    nc.gpsimd.partition_broadcast(rb[0:64, 512:577], rr[0:1, 512:577])
                    fins[hh] = (avs[hh], ps2, rb)
                return fins

            def omult(b, hh, fin, half):
                # O = O'/denominator, reading O' straight out of PSUM
                ps1, ps2, rb = fin
                pr = hh % 2
                qt = hh // 2
                qoff = (hh % 2) * 64
                if half == 0:
                    if not OMULT_FROM_PSUM:
                        ost = dnp.tile([64, 512], f32, tag="ost", name="ost")
                        nc.vector.tensor_copy(ost[0:64, 0:512], ps1[0:64, 0:512])
                        src = ost[0:64, 0:512]
                    else:
                        src = ps1[0:64, 0:512]
                    nc.vector.tensor_tensor(
                        o[qt][qoff : qoff + 64, b * N : b * N + 512],
                        src,
                        rb[0:64, 0:512],
                        op=Alu.mult,
                    )
                else:
                    if not OMULT_FROM_PSUM:
                        ost = dnp.tile([64, 512], f32, tag="ost2", name="ost2")
                        nc.vector.tensor_copy(
                            ost[0:64, 0:65], ps2[0:64, 512 * pr : 512 * pr + 65]
                        )
                        src = ost[0:64, 0:65]
                    else:
                        src = ps2[0:64, 512 * pr : 512 * pr + 65]
                    nc.vector.tensor_tensor(
                        o[qt][qoff : qoff + 64, b * N + 512 : b * N + N],
                        src,
                        rb[0:64, 512:577],
                        op=Alu.mult,
                    )

            # proj token chunks, batch-aligned
            PFREE0 = [(0, 512), (512, 65)]           # batch 0 tokens
            PFREE1 = [(577, 512), (1089, 65)]        # batch 1 tokens

            # ------------- pipelined emission schedule ---------------------
            def qk_chunk(t, ci, eng):
                def f():
                    f0, fsz = TFREE[ci]
                    qk_chunk_emit(t, f0, fsz, eng)
                return f

            def v_half(bb, j, half):
                def f():
                    v_half_emit(bb, j, half)
                return f

            def pj(t, ci, eng="dve"):
                def f():
                    f0, fsz = PFREE0[ci]
                    proj_group(t, f0, fsz, eng)
                return f

            def pt_dma():
                def f():
                    nc.sync.dma_start(
                        ptall[:, :, :], projT[:, :].rearrange("(i p) n -> p i n", p=P)
                    )
                return f

            # b1-proj partial accumulation: during iteration 11, o[0..4]'s b1
            # columns are complete (pairs 6..10), so tiles t=0,1 can run
            # their first five contraction steps early; only the ki=5 matmul
            # + evict + DMA remain for the tail. The psum tiles stay live
            # (no other ps_s user between the partials and the finish).
            pre_ps = {}

            def pj_b1_partial(t):
                def f():
                    ps = ps_s.tile([P, 1024], f32, tag="ps_s", name="psmm")
                    pre_ps[t] = ps
                    for ci, (f0, fsz) in enumerate(PFREE1):
                        for ki in range(5):
                            nc.tensor.matmul(
                                ps[:, 577 * ci : 577 * ci + fsz],
                                pt[ki][:, P * t : P * (t + 1)],
                                o[ki][:, f0 : f0 + fsz],
                                start=(ki == 0),
                                stop=False,
                            )
                return f

            # The c2 (b1-token) qk chunks have late deadlines -- qk[t] b1
            # columns are first read by the S of pair 6+qt, i.e. iteration
            # 5+qt -- so they move out of the ACT-crowded early iterations.
            fill = {
                0: [qk_chunk(2, 0, "act"), qk_chunk(8, 0, "act"),
                    qk_chunk(2, 1, "act"), qk_chunk(8, 1, "act")],
                1: [qk_chunk(3, 0, "act"), qk_chunk(9, 0, "act"),
                    qk_chunk(3, 1, "act"), qk_chunk(9, 1, "act")],
                2: [qk_chunk(4, 0, "act"), qk_chunk(10, 0, "act"),
                    qk_chunk(4, 1, "act"), qk_chunk(10, 1, "act")],
                3: [qk_chunk(5, 0, "act"), qk_chunk(11, 0, "act"),
                    qk_chunk(5, 1, "act"), qk_chunk(11, 1, "act")],
                4: [pt_dma(), v_half(1, 0, 0), v_half(1, 0, 1),
                    v_half(1, 1, 0), v_half(1, 1, 1), v_half(1, 2, 0)],
                5: [v_half(1, 2, 1), v_half(1, 3, 0), v_half(1, 3, 1),
                    v_half(1, 4, 0), v_half(1, 4, 1)],
                6: [qk_chunk(2, 2, "act"), qk_chunk(8, 2, "act"),
                    pj(0, 0, "act"), pj(0, 1, "act")],
                7: [qk_chunk(3, 2, "act"), qk_chunk(9, 2, "act"),
                    pj(1, 0, "act"), pj(1, 1, "act")],
                8: [qk_chunk(4, 2, "act"), qk_chunk(10, 2, "act"),
                    pj(2, 0, "act"), pj(2, 1, "act")],
                9: [qk_chunk(5, 2, "act"), qk_chunk(11, 2, "act"),
                    pj(3, 0, "act"), pj(3, 1, "act")],
                10: [pj(4, 0, "act"), pj(4, 1, "act")],
                11: [pj(5, 0, "act"), pj(5, 1, "act"),
                     pj_b1_partial(0), pj_b1_partial(1)],
            }

            # ---- lead-in: qk 0/6, then pair-0 S interleaved with b0 V ----
            qk_group(0)
            qk_group(6)
            pairs = [(b, h0) for b in range(BPC) for h0 in range(0, 12, 2)]
            st = s_prologue(*pairs[0])
            for j in range(5):
                s_step(*pairs[0], st, j)
                v_half_emit(0, j, 0)
                v_half_emit(0, j, 1)
            s_epilogue(st)
            qk_group(1)
            qk_group(7)
            mult_phase(st)
            cur = {0: st}
            for k in range(12):
                b, h0 = pairs[k]
                staP = cur[k][0]
                fq = list(fill[k])
                avs = {hh: av_alloc(hh) for hh in (h0, h0 + 1)}
                if k + 1 < 12:
                    nb, nh0 = pairs[k + 1]
                    cur[k + 1] = s_prologue(nb, nh0)
                    # interleave: S of pair k+1 with AV of pair k, per nk tile
                    for j in range(5):
                        s_step(nb, nh0, cur[k + 1], j)
                        for hh in (h0, h0 + 1):
                            av_step(b, hh, avs[hh], staP, j)
                    s_epilogue(cur[k + 1])
                else:
                    for j in range(5):
                        for hh in (h0, h0 + 1):
                            av_step(b, hh, avs[hh], staP, j)
                # mult of pair k+1 goes FIRST on the DVE queue: it gates the
                # next iteration's AV matmuls, while the denominator chain
                # below only gates this pair's o-mults
                if k + 1 < 12:
                    mult_phase(cur[k + 1])
                drs = {hh: av_evict1(avs[hh]) for hh in (h0, h0 + 1)}
                fins = av_pass2(b, h0, avs, drs, staP)
                while fq:           # leftovers (before the omults: proj
                    fq.pop(0)()     # must not depend on this pair's o writes)
                for hh in (h0, h0 + 1):
                    omult(b, hh, fins[hh], 0)
                for hh in (h0, h0 + 1):
                    omult(b, hh, fins[hh], 1)
                del cur[k]
            # ----- remaining output projection: one DMA per outc tile ------
            for t in range(6):
                ott = oevp.tile([P, N], bf16, tag="oevt", name="oevt")
                if t in pre_ps:
                    # finish the pre-accumulated tiles: ki=5 only
                    ps = pre_ps[t]
                    for ci, (f0, fsz) in enumerate(PFREE1):
                        nc.tensor.matmul(
                            ps[:, 577 * ci : 577 * ci + fsz],
                            pt[5][:, P * t : P * (t + 1)],
                            o[5][:, f0 : f0 + fsz],
                            start=False,
                            stop=True,
                        )
                    for ci, (f0, fsz) in enumerate(PFREE1):
                        nc.scalar.activation(
                            ott[:, f0 - N : f0 - N + fsz],
                            ps[:, 577 * ci : 577 * ci + fsz],
                            Act.Identity,
                            bias=pb[:, t : t + 1],
                        )
                else:
                    for (f0, fsz) in PFREE1:
                        ps = ps_s.tile([P, 1024], f32, tag="ps_s", name="psmm")
                        for ki in range(6):
                            nc.tensor.matmul(
                                ps[:, 0:fsz],
                                pt[ki][:, P * t : P * (t + 1)],
                                o[ki][:, f0 : f0 + fsz],
                                start=(ki == 0),
                                stop=(ki == 5),
                            )
                        nc.scalar.activation(
                            ott[:, f0 - N : f0 - N + fsz], ps[:, 0:fsz], Act.Identity,
                            bias=pb[:, t : t + 1],
                        )
                nc.sync.dma_start(out[P * t : P * (t + 1), N:NT], ott[:, 0:N])

    nc.compile()
    return nc


def _get_nc():
    if "nc" not in _CACHE:
        _CACHE["nc"] = _build()
    return _CACHE["nc"]


def make_in_maps(x, rel_pos_bias, qkv_w, q_bias, v_bias, proj_w, proj_b):
    bf = ml_dtypes.bfloat16
    x = np.asarray(x, dtype=np.float32)
    rel_pos_bias = np.asarray(rel_pos_bias, dtype=np.float32)
    qkv_w = np.asarray(qkv_w, dtype=np.float32)
    q_bias = np.asarray(q_bias, dtype=np.float32)
    v_bias = np.asarray(v_bias, dtype=np.float32)
    proj_w = np.asarray(proj_w, dtype=np.float32)
    proj_b = np.asarray(proj_b, dtype=np.float32)

    wqkvT = np.ascontiguousarray(qkv_w.T).astype(bf)                    # [768, 2304]
    # contiguous per-block weight arrays (full-bandwidth DMA reads)
    wq0d = np.ascontiguousarray(wqkvT[:, 0:P])
    wk0d = np.ascontiguousarray(wqkvT[:, 6 * P : 7 * P])
    wqrd = np.ascontiguousarray(wqkvT[:, P : 6 * P])
    wkrd = np.ascontiguousarray(wqkvT[:, 7 * P : 2 * C])
    wvd = np.ascontiguousarray(wqkvT[:, 2 * C : 3 * C])
    qbias = np.ascontiguousarray((q_bias * 0.125).reshape(6, P).T)      # [128, 6]
    # v_bias folds into the projection bias: attn rows sum to 1, so
    # out = attn @ (x Wv^T + v_b) Wp^T + p_b = attn x Wv^T Wp^T + (p_b + Wp v_b)
    pbias_full = proj_b + proj_w @ v_bias
    # exp of the transposed rel-pos bias: applied multiplicatively after exp(S);
    # rows padded 577->640 (5*128) so each head loads as a single strided DMA
    relbT = np.zeros((H, 640, N), dtype=bf)
    relbT[:, :N, :] = np.exp(rel_pos_bias[0].transpose(0, 2, 1)).astype(bf)
    projT = np.ascontiguousarray(proj_w.T).astype(bf)                   # [768, 768]
    pbias = np.ascontiguousarray(pbias_full.reshape(6, P).T)            # [128, 6]

    in_maps = []
    for c in range(NCORES):
        xT = x[BPC * c : BPC * (c + 1)].reshape(NT, C).T                # [768, 1154]
        xTa = np.ascontiguousarray(xT[:, 0:512]).astype(bf)
        xTb = np.ascontiguousarray(xT[:, 512:NT]).astype(bf)
        in_maps.append(
            dict(
                xTa=xTa,
                xTb=xTb,
                wq0d=wq0d,
                wk0d=wk0d,
                wqrd=wqrd,
                wkrd=wkrd,
                wvd=wvd,
                qbias=qbias,
                relbT=relbT,
                projT=projT,
                pbias=pbias,
            )
        )
    return in_maps


def kernel(x, rel_pos_bias, qkv_w, q_bias, v_bias, proj_w, proj_b):
    from concourse import bass_utils

    in_maps = make_in_maps(x, rel_pos_bias, qkv_w, q_bias, v_bias, proj_w, proj_b)
    nc = _get_nc()
    res = bass_utils.run_bass_kernel_spmd(nc, in_maps, core_ids=list(range(NCORES)))
    outs = []
    for c in range(NCORES):
        oT = res.results[c]["out"]                                      # [768, 1154] bf16
        outs.append(np.ascontiguousarray(oT.T).astype(np.float32).reshape(BPC, N, C))
    return np.concatenate(outs, axis=0)
